# revision 57
# baseline (speedup 1.0000x reference)
"""MLA attention TRN2 kernel: 4-way data-parallel (sequences) x 2-way
tensor-parallel (heads). Each core: 1 sequence (1024 tokens), 8 heads.

v2: all matmuls in bf16 (fp32r runs in fp32-HIGH PE mode at ~1.3 cyc/row
with slow non-FWL weight loads; bf16 gets 1.0 cyc/row + fast weight
load). RoPE done as batched full-width DVE muls + PE signed-swap
matmuls instead of 32-row DVE ops. A warmup matmul stream at program
start flips the PE HAM clock gate to 2.4 GHz while the x DMA streams.
Weights are pre-laid out on host into SBUF tile shapes so all weight
DMAs are contiguous.

Layout convention: features on partitions, tokens on free axis; scores
computed transposed [k, q] so softmax sums use PE ones-matmuls and no
transposes are needed anywhere. Softmax denominator folded into avT via
per-(head, qblock-pair) broadcast multiply.
"""
import sys
sys.path.insert(0, '/opt/trn_rl_repo')

import math
import numpy as np

import concourse.bass as bass
import concourse.tile as tile
from concourse.tile_rust import add_dep_helper
from concourse import bacc, mybir

BF = mybir.dt.bfloat16
F32 = mybir.dt.float32
AF = mybir.ActivationFunctionType

H = 16
NH = 8            # heads per core
NOPE = 128
ROPE = 64
VD = 128
HID = 2048
QLR = 1536
KVLR = 512
B = 4
S = 1024
EPS = 1e-6
SCALE = 1.0 / math.sqrt(NOPE + ROPE)
NEG = -1.0e30

HID_T = HID // 128    # 16
QLR_T = QLR // 128    # 12
KVLR_T = KVLR // 128  # 4
QLOC = QLR_T // 2     # 6 u strips computed locally (feature-split pair)
KLOC = KVLR_T // 2    # 2 ckv strips computed locally
CC_GROUPS = [[0, 1], [2, 3], [4, 5], [6, 7]]
QF = NH * (NOPE + ROPE)   # 1536 q features per core
QF_T = QF // 128          # 12 (chunks 0..7 nope, 8..11 rope)
NTC = S // 128            # 8 token chunks
QB = 256                  # query block
NQB = S // QB             # 4
N_WARM = 24               # warmup matmuls (keep HAM busy >3.4us)


def build_nc():
    nc = bacc.Bacc("TRN2", target_bir_lowering=False, debug=False, num_devices=8)

    xp = nc.dram_tensor("xp", [128, HID_T, S], BF, kind="ExternalInput")
    wqa = nc.dram_tensor("wqa", [QLOC, 128, HID_T, 128], BF, kind="ExternalInput")
    wqb = nc.dram_tensor("wqb", [QF_T, 128, QLR_T, 128], BF, kind="ExternalInput")
    wkva = nc.dram_tensor("wkva", [KLOC, 128, HID_T, 128], BF, kind="ExternalInput")
    wkvar = nc.dram_tensor("wkvar", [128, HID_T, ROPE], BF, kind="ExternalInput")
    wkb = nc.dram_tensor("wkb", [KVLR_T, 128, NH * NOPE], BF, kind="ExternalInput")
    wvb = nc.dram_tensor("wvb", [KVLR_T, 128, NH * VD], BF, kind="ExternalInput")
    wo = nc.dram_tensor("wo", [4, 128, NH, 512], BF, kind="ExternalInput")
    c128 = nc.dram_tensor("c128", [128, S], BF, kind="ExternalInput")
    s128 = nc.dram_tensor("s128", [128, S], BF, kind="ExternalInput")
    psw = nc.dram_tensor("psw", [128, 128], BF, kind="ExternalInput")
    id128 = nc.dram_tensor("id128", [128, 128], BF, kind="ExternalInput")
    masks = nc.dram_tensor("masks", [2, 128, QB], F32, kind="ExternalInput")
    ones_col = nc.dram_tensor("ones_col", [128, 1], BF, kind="ExternalInput")
    ones_row = nc.dram_tensor("ones_row", [1, 128], BF, kind="ExternalInput")
    ones_sq = nc.dram_tensor("ones_sq", [128, 128], BF, kind="ExternalInput")
    out = nc.dram_tensor("out", [S, HID], BF, kind="ExternalOutput")
    # pair-exchange scratch: u strips streamed as 3 gathers of 2 strips each
    # (flat row-major; last one carries the z2q partial row), ckv as one.
    du_loc = [nc.dram_tensor(f"du_loc{i}", [256, S],
                             BF, kind="Internal") for i in range(3)]
    du_gat = [nc.dram_tensor(f"du_gat{i}", [2, 256, S],
                             BF, kind="Internal") for i in range(3)]
    dz_loc = nc.dram_tensor("dz_loc", [1, S], BF, kind="Internal")
    dz_gat = nc.dram_tensor("dz_gat", [2, 1, S], BF, kind="Internal")
    dk_loc = nc.dram_tensor("dk_loc", [KLOC * 128 + 1, S], BF, kind="Internal")
    dk_gat = nc.dram_tensor("dk_gat", [2, KLOC * 128 + 1, S], BF, kind="Internal")

    with tile.TileContext(nc, pool_alloc_mode="queue") as tc:
        build_body(tc, xp=xp, wqa=wqa, wqb=wqb, wkva=wkva, wkvar=wkvar,
                   wkb=wkb, wvb=wvb, wo=wo, c128=c128, s128=s128, psw=psw,
                   id128=id128, masks=masks, ones_col=ones_col,
                   ones_row=ones_row, ones_sq=ones_sq, out=out,
                   du_loc=du_loc, du_gat=du_gat, dk_loc=dk_loc, dk_gat=dk_gat,
                   dz_loc=dz_loc, dz_gat=dz_gat)
    nc.compile()
    return nc


def build_body(tc, *, xp, wqa, wqb, wkva, wkvar, wkb, wvb, wo, c128, s128,
               psw, id128, masks, ones_col, ones_row, ones_sq, out,
               du_loc, du_gat, dk_loc, dk_gat, dz_loc, dz_gat):
    from contextlib import ExitStack
    nc = tc.nc

    with (
        tc.tile_pool(name="const", bufs=1) as pconst,
        tc.tile_pool(name="ckv", bufs=1) as pckv,
    ):
        ones_c = pconst.tile([128, 1], BF, name="ones_c")
        nc.sync.dma_start(ones_c[:], ones_col[:])
        ones_r = pconst.tile([1, 128], BF, name="ones_r")
        nc.sync.dma_start(ones_r[:], ones_row[:])
        ones_s = pconst.tile([128, 128], BF, name="ones_s")
        nc.sync.dma_start(ones_s[:], ones_sq[:])
        cos_sb = pconst.tile([128, S], BF, name="cos_sb")
        nc.sync.dma_start(cos_sb[:], c128[:])
        sin_sb = pconst.tile([128, S], BF, name="sin_sb")
        nc.sync.dma_start(sin_sb[:], s128[:])
        psw_sb = pconst.tile([128, 128], BF, name="psw_sb")
        nc.sync.dma_start(psw_sb[:], psw[:])
        id_sb = pconst.tile([128, 128], BF, name="id_sb")
        nc.sync.dma_start(id_sb[:], id128[:])
        mask_sb = []
        for i in range(2):
            m = pconst.tile([128, QB], F32, name=f"mask{i}")
            nc.sync.dma_start(m[:], masks[i])
            mask_sb.append(m)
        kpe2 = pconst.tile([128, S], BF, name="kpe2")
        eps_t = pconst.tile([1, 1], F32, name="eps_t")
        nc.vector.memset(eps_t[:], EPS)

        ckv = [pckv.tile([128, S], BF, name=f"ckv{i}") for i in range(KVLR_T)]

        # ---- HAM warmup: keep PE busy while x/wqa DMAs stream ----
        with tc.tile_pool(name="pwarm", bufs=1, space="PSUM") as ppw:
            wps = ppw.tile([128, 512], F32, name="warm_ps")
            for i in range(N_WARM):
                nc.tensor.matmul(wps[:], ones_s[:], cos_sb[:, 0:512],
                                 start=True, stop=True, skip_group_check=True)

        # ============ Phases: q_a -> kv_a (norms overlap) -> q_b ============
        es_u = ExitStack()
        pu = es_u.enter_context(tc.tile_pool(name="u", bufs=1))
        es_qbc = ExitStack()
        pqbc = es_qbc.enter_context(tc.tile_pool(name="qbc", bufs=1))
        qbc = [pqbc.tile([128, 512], BF, name=f"qbcn{i}") for i in range(2)]
        es_x = ExitStack()
        px = es_x.enter_context(tc.tile_pool(name="xsb", bufs=1))

        xsb = px.tile([128, HID_T, S], BF, name="xsb")
        prev_x = {0: None, 1: None}
        for kb in range(4):
            eng = nc.gpsimd if kb % 2 == 0 else nc.sync
            dma = eng.dma_start(xsb[:, kb * 4:(kb + 1) * 4, :],
                                xp[:, kb * 4:(kb + 1) * 4, :])
            if prev_x[kb % 2] is not None:
                add_dep_helper(dma.ins, prev_x[kb % 2].ins, sync=True,
                               reason="serialize x waves so early tiles land first")
            prev_x[kb % 2] = dma
        u = [pu.tile([128, S], BF, name=f"u{i}") for i in range(QLR_T)]

        with (
            tc.tile_pool(name="wstripA", bufs=4) as pwA,
            tc.tile_pool(name="sqA", bufs=4) as psq,
            tc.tile_pool(name="normA", bufs=2) as pnorm,
            tc.tile_pool(name="ppmain", bufs=4, space="PSUM") as ppm,
            tc.tile_pool(name="ppz2", bufs=2, space="PSUM") as ppz2,
            tc.tile_pool(name="ppbc", bufs=2, space="PSUM") as ppbc,
        ):
            # ---- kv_a first: local 2 ckv strips + k_pe strip (both cores);
            # its small gather then hides under q_a ----
            z2k = [ppz2.tile([1, 512], F32, name=f"z2k{h}", tag="z2")
                   for h in range(2)]
            kv4 = psq.tile([64, S], BF, name="kv4", tag="kv4", bufs=1)
            ckstage = [pu.tile([128, S], BF, name=f"ckst{i}")
                       for i in range(KLOC)]
            zkstage = pnorm.tile([1, S], BF, name="zkstage", tag="zst", bufs=1)
            k_dmas = []
            pend = []
            with nc.named_scope("kv_a"):
                for m in range(KLOC):
                    ws = pwA.tile([128, HID_T, 128], BF, name=f"wkva_s{m}",
                                  tag="wstrip")
                    nc.sync.dma_start(ws[:], wkva[m])
                    for half in range(2):
                        sl = slice(half * 512, (half + 1) * 512)
                        ps = ppm.tile([128, 512], F32, name=f"pskv{m}_{half}",
                                      tag="main")
                        for ko in range(HID_T):
                            mmk = nc.tensor.matmul(
                                ps[:], ws[:, ko, :], xsb[:, ko, sl],
                                start=(ko == 0), stop=(ko == HID_T - 1))
                            if m == 0 and half == 0 and ko == 0:
                                gate_kva = mmk
                        cpk = nc.scalar.copy(ckstage[m][:, sl], ps[:])
                        sq = psq.tile([128, 512], BF, name=f"sqk{m}_{half}",
                                      tag="sq")
                        nc.scalar.activation(sq[:], ps[:], AF.Square)
                        for fn in pend:
                            fn()
                        pend = []
                        pend.append(
                            lambda sq=sq, half=half, m=m: nc.tensor.matmul(
                                z2k[half][:], ones_c[:], sq[:],
                                start=(m == 0), stop=(m == KLOC - 1),
                                skip_group_check=True))
                    k_dmas.append(nc.gpsimd.dma_start(
                        dk_loc[m * 128:(m + 1) * 128, :], ckstage[m][:]))
                for fn in pend:
                    fn()
                pend = []
                for half in range(2):
                    sl = slice(half * 512, (half + 1) * 512)
                    nc.scalar.copy(zkstage[:, sl], z2k[half][:])
                k_dmas.append(
                    nc.gpsimd.dma_start(dk_loc[KLOC * 128:, :], zkstage[:]))
                cck = nc.gpsimd.collective_compute(
                    "AllGather", mybir.AluOpType.bypass,
                    replica_groups=CC_GROUPS,
                    ins=[dk_loc[:]], outs=[dk_gat[:]])
                for dma in k_dmas:
                    add_dep_helper(cck.ins, dma.ins, sync=True,
                                   reason="gather after ckv stage DMAs")
                for ko in range(KVLR_T):
                    g, i = ko // KLOC, ko % KLOC
                    dma = nc.gpsimd.dma_start(
                        ckv[ko][:], dk_gat[g, i * 128:(i + 1) * 128, :])
                    add_dep_helper(dma.ins, cck.ins, sync=True,
                                   reason="read ckv after gather")
                zkg = [pnorm.tile([1, S], BF, name=f"zkg{g}", tag=f"zkg{g}",
                                  bufs=1) for g in range(2)]
                for g in range(2):
                    dma = nc.gpsimd.dma_start(zkg[g][:],
                                              dk_gat[g, KLOC * 128:, :])
                    add_dep_helper(dma.ins, cck.ins, sync=True,
                                   reason="read z2k after gather")
                zkt = pnorm.tile([1, S], F32, name="zkt", tag="zkt", bufs=1)
                nc.vector.tensor_add(zkt[:], zkg[0][:], zkg[1][:])

            # ---- q_a: local 6 u strips; streamed pairwise AllGathers ----
            z2q = [ppz2.tile([1, 512], F32, name=f"z2q{h}", tag="z2")
                   for h in range(2)]
            ustage = [pu.tile([128, S], BF, name=f"ust{i}")
                      for i in range(QLOC)]
            zqstage = pnorm.tile([1, S], BF, name="zqstage", tag="zst", bufs=1)
            u_dmas = []
            zqg = [pqbc.tile([1, S], BF, name=f"zqg{g}") for g in range(2)]
            zqt = pqbc.tile([1, S], F32, name="zqt")

            def u_gather(cc_idx):
                # gather strips (2*cc_idx, 2*cc_idx+1)
                cc = nc.gpsimd.collective_compute(
                    "AllGather", mybir.AluOpType.bypass,
                    replica_groups=CC_GROUPS,
                    ins=[du_loc[cc_idx][:]], outs=[du_gat[cc_idx][:]])
                for dma in u_dmas:
                    add_dep_helper(cc.ins, dma.ins, sync=True,
                                   reason="gather after u stage DMAs")
                u_dmas.clear()
                for g in range(2):
                    for i in range(2):
                        ko = g * QLOC + cc_idx * 2 + i
                        dma = nc.gpsimd.dma_start(
                            u[ko][:],
                            du_gat[cc_idx][g, i * 128:(i + 1) * 128, :])
                        add_dep_helper(dma.ins, cc.ins, sync=True,
                                       reason="read u after gather")

            with nc.named_scope("q_a"):
                for m in range(QLOC):
                    ws = pwA.tile([128, HID_T, 128], BF, name=f"wqa_s{m}",
                                  tag="wstrip")
                    dma = nc.sync.dma_start(ws[:], wqa[m])
                    add_dep_helper(dma.ins, gate_kva.ins, sync=True,
                                   reason="stage wqa DMA after kv_a starts")
                    for half in range(2):
                        sl = slice(half * 512, (half + 1) * 512)
                        ps = ppm.tile([128, 512], F32, name=f"psu{m}_{half}",
                                      tag="main")
                        for ko in range(HID_T):
                            mmq = nc.tensor.matmul(
                                ps[:], ws[:, ko, :], xsb[:, ko, sl],
                                start=(ko == 0), stop=(ko == HID_T - 1))
                            if m == 0 and half == 0 and ko == 0:
                                gate_qa = mmq
                        cp = nc.scalar.copy(ustage[m][:, sl], ps[:])
                        sq = psq.tile([128, 512], BF, name=f"squ{m}_{half}",
                                      tag="sq")
                        nc.scalar.activation(sq[:], ps[:], AF.Square)
                        for fn in pend:
                            fn()
                        pend = []
                        pend.append(
                            lambda sq=sq, half=half, m=m: nc.tensor.matmul(
                                z2q[half][:], ones_c[:], sq[:],
                                start=(m == 0), stop=(m == QLOC - 1),
                                skip_group_check=True))
                    u_dmas.append(nc.gpsimd.dma_start(
                        du_loc[m // 2][(m % 2) * 128:(m % 2 + 1) * 128, :],
                        ustage[m][:]))
                    if m in (1, 3, 5):
                        u_gather(m // 2)
                for fn in pend:
                    fn()
                pend = []
                # z2q partials ride their own tiny gather so the u strips
                # aren't held back behind the z2q psum stop
                for half in range(2):
                    sl = slice(half * 512, (half + 1) * 512)
                    nc.scalar.copy(zqstage[:, sl], z2q[half][:])
                zdma = nc.gpsimd.dma_start(dz_loc[:], zqstage[:])
                ccz = nc.gpsimd.collective_compute(
                    "AllGather", mybir.AluOpType.bypass,
                    replica_groups=CC_GROUPS,
                    ins=[dz_loc[:]], outs=[dz_gat[:]])
                add_dep_helper(ccz.ins, zdma.ins, sync=True,
                               reason="z gather after stage")
                for g in range(2):
                    dma = nc.gpsimd.dma_start(zqg[g][:], dz_gat[g])
                    add_dep_helper(dma.ins, ccz.ins, sync=True,
                                   reason="read z2q after gather")
                nc.vector.tensor_add(zqt[:], zqg[0][:], zqg[1][:])

            # ---- k_pe strip (independent of the gathers: fills the
            # cck latency window right after q_a) ----
            with nc.named_scope("kv_rope"):
                ws = pwA.tile([128, HID_T, ROPE], BF, name="wkva_s4",
                              tag="wstrip")
                nc.sync.dma_start(ws[:], wkvar[:])
                for half in range(2):
                    sl = slice(half * 512, (half + 1) * 512)
                    ps = ppm.tile([128, 512], F32, name=f"pskvr_{half}",
                                  tag="main")
                    for ko in range(HID_T):
                        nc.tensor.matmul(
                            ps[:ROPE, :], ws[:, ko, :], xsb[:, ko, sl],
                            start=(ko == 0), stop=(ko == HID_T - 1))
                    nc.scalar.copy(kv4[:, sl], ps[:ROPE, :])

            # ---- ckv rmsnorm scale + k_pe RoPE (PE signed-swap) ----
            with nc.named_scope("ckv_norm"):
                for half in range(2):
                    sl = slice(half * 512, (half + 1) * 512)
                    rt = pnorm.tile([1, 512], F32, name=f"krt{half}", tag="rt", bufs=1)
                    nc.scalar.activation(rt[:], zkt[:, sl], AF.Sqrt,
                                         scale=1.0 / KVLR, bias=eps_t[:])
                    rr = pnorm.tile([1, 512], BF, name=f"krr{half}", tag="rr", bufs=1)
                    with nc.allow_low_precision(reason="bf16 rms factor"):
                        nc.vector.reciprocal(rr[:], rt[:])
                    pb = ppbc.tile([128, 512], F32, name=f"kpb{half}", tag="bc")
                    nc.tensor.matmul(pb[:], ones_r[:], rr[:])
                    bc = pnorm.tile([128, 512], BF, name=f"kbc{half}", tag="bc")
                    nc.scalar.copy(bc[:], pb[:])
                    for m in range(KVLR_T):
                        nc.vector.tensor_mul(ckv[m][:, sl], ckv[m][:, sl],
                                             bc[:])

                # k_pe RoPE: elementwise cos/sin muls then PE pair-swap
                kta = pnorm.tile([64, S], BF, name="kta", tag="r64a", bufs=1)
                nc.vector.tensor_mul(kta[:], kv4[:], cos_sb[0:64, :])
                ktb = pnorm.tile([64, S], BF, name="ktb", tag="r64b", bufs=1)
                nc.vector.tensor_mul(ktb[:], kv4[:], sin_sb[0:64, :])
                for half in range(2):
                    sl = slice(half * 512, (half + 1) * 512)
                    pk = ppbc.tile([64, 512], F32, name=f"pkpe{half}", tag="bc")
                    nc.tensor.matmul(pk[:], psw_sb[0:64, 0:64], ktb[:, sl],
                                     start=True, stop=False)
                    nc.tensor.matmul(pk[:], id_sb[0:64, 0:64], kta[:, sl],
                                     start=False, stop=True)
                    nc.scalar.copy(kpe2[0:64, sl], pk[:])
                nc.gpsimd.dma_start(kpe2[64:128, :], kpe2[0:64, :])

        es_x.close()

        # ---- v_b early: covers the tail of the u AllGather ----
        es_v = ExitStack()
        pv = es_v.enter_context(tc.tile_pool(name="v", bufs=1, side="right"))
        v_sb = [[pv.tile([128, 512], BF, name=f"v{g}_{t}")
                 for t in range(NTC)] for g in range(2)]
        with (
            tc.tile_pool(name="wvb", bufs=1) as pwvb,
            tc.tile_pool(name="ppv", bufs=4, space="PSUM") as ppv,
        ):
            wvb_sb = [pwvb.tile([128, NH * VD], BF, name=f"wvb{i}")
                      for i in range(KVLR_T)]
            for ko in range(KVLR_T):
                dma = nc.sync.dma_start(wvb_sb[ko][:], wvb[ko])
                add_dep_helper(dma.ins, gate_qa.ins, sync=True,
                               reason="stage wvb DMA after q_a starts")
            with nc.named_scope("v_b"):
                for g in range(2):
                    for t in range(NTC):
                        ps = ppv.tile([128, 512], F32,
                                      name=f"psv{g}_{t}", tag="v")
                        for kk in range(KVLR_T):
                            nc.tensor.matmul(
                                ps[:],
                                ckv[kk][:, t * 128:(t + 1) * 128],
                                wvb_sb[kk][:, g * 512:(g + 1) * 512],
                                start=(kk == 0),
                                stop=(kk == KVLR_T - 1))
                        nc.scalar.copy(v_sb[g][t][:], ps[:])

        # ---- k_nope for all heads: fills the u-AllGather latency window ----
        es_kn = ExitStack()
        pkn = es_kn.enter_context(tc.tile_pool(name="kn", bufs=1, side="right"))
        kn_all = [pkn.tile([128, S], BF, name=f"kn{h}") for h in range(NH)]
        es_wkb = ExitStack()
        pwkb = es_wkb.enter_context(tc.tile_pool(name="wkb", bufs=1))
        wkb_sb = [pwkb.tile([128, NH * NOPE], BF, name=f"wkb{i}")
                  for i in range(KVLR_T)]
        for ko in range(KVLR_T):
            dma = nc.sync.dma_start(wkb_sb[ko][:], wkb[ko])
            add_dep_helper(dma.ins, gate_qa.ins, sync=True,
                           reason="stage wkb DMA after q_a starts")
        with tc.tile_pool(name="ppkn", bufs=2, space="PSUM") as ppkn:
            with nc.named_scope("k_b"):
                for hh in range(NH):
                    for half in range(2):
                        sl = slice(half * 512, (half + 1) * 512)
                        ps = ppkn.tile([128, 512], F32,
                                       name=f"pskn{hh}_{half}", tag="kn")
                        for kk in range(KVLR_T):
                            nc.tensor.matmul(
                                ps[:], wkb_sb[kk][:, hh * 128:(hh + 1) * 128],
                                ckv[kk][:, sl],
                                start=(kk == 0), stop=(kk == KVLR_T - 1))
                        nc.scalar.copy(kn_all[hh][:, sl], ps[:])
        es_wkb.close()

        # ---- u rmsnorm factor: placed after kn so its PE broadcast never
        # blocks v_b/k_b on the z2q gather chain ----
        with (
            tc.tile_pool(name="normB", bufs=1) as pn2,
            tc.tile_pool(name="ppbc2", bufs=2, space="PSUM") as ppbc2,
        ):
            with nc.named_scope("u_norm"):
                for half in range(2):
                    sl = slice(half * 512, (half + 1) * 512)
                    rt = pn2.tile([1, 512], F32, name=f"qrt{half}")
                    nc.scalar.activation(rt[:], zqt[:, sl], AF.Sqrt,
                                         scale=1.0 / QLR, bias=eps_t[:])
                    rr = pn2.tile([1, 512], BF, name=f"qrr{half}")
                    with nc.allow_low_precision(reason="bf16 rms factor"):
                        nc.vector.reciprocal(rr[:], rt[:])
                    pb = ppbc2.tile([128, 512], F32, name=f"qpb{half}", tag="bc")
                    nc.tensor.matmul(pb[:], ones_r[:], rr[:])
                    nc.scalar.copy(qbc[half][:], pb[:])

        # ---- q_b: qT = wqbT.T @ u -> [1536, S]; rope tiles first ----
        es_qT = ExitStack()
        pqT = es_qT.enter_context(tc.tile_pool(name="qT", bufs=1, side="right"))
        qT = [pqT.tile([128, S], BF, name=f"qTt{i}") for i in range(QF_T)]
        with (
            tc.tile_pool(name="wstripB", bufs=4) as pwB,
            tc.tile_pool(name="ropeB", bufs=2) as prope,
            tc.tile_pool(name="ppmainB", bufs=6, space="PSUM") as ppmB,
            tc.tile_pool(name="pprope", bufs=2, space="PSUM") as ppr,
        ):
            def rope_xform(m):
                # rope transform in place: qT[m] <- ta + Psw @ tb
                ta = prope.tile([128, S], BF, name=f"rta{m}", tag="ta")
                nc.vector.tensor_mul(ta[:], qT[m][:], cos_sb[:])
                tb = prope.tile([128, S], BF, name=f"rtb{m}", tag="tb")
                nc.vector.tensor_mul(tb[:], qT[m][:], sin_sb[:])
                for half in range(2):
                    sl = slice(half * 512, (half + 1) * 512)
                    pr = ppr.tile([128, 512], F32, name=f"psr{m}_{half}",
                                  tag="rope")
                    nc.tensor.matmul(pr[:], psw_sb[:], tb[:, sl],
                                     start=True, stop=False)
                    nc.tensor.matmul(pr[:], id_sb[:], ta[:, sl],
                                     start=False, stop=True)
                    nc.scalar.copy(qT[m][:, sl], pr[:])

            with nc.named_scope("q_b"):
                for mi, m in enumerate(list(range(NH, QF_T)) + list(range(NH))):
                    ws = pwB.tile([128, QLR_T, 128], BF, name=f"wqb_s{m}",
                                  tag="wstripB")
                    dma = nc.sync.dma_start(ws[:], wqb[m])
                    add_dep_helper(dma.ins, gate_kva.ins, sync=True,
                                   reason="stage wqb DMA after kv_a starts")
                    # contract over ko with the last-gathered strips at the
                    # end, so early q_b groups don't wait on the final CC
                    ko_order = [0, 1, 2, 3, 6, 7, 8, 9, 4, 5, 10, 11]
                    for half in range(2):
                        sl = slice(half * 512, (half + 1) * 512)
                        ps = ppmB.tile([128, 512], F32, name=f"psq{m}_{half}",
                                       tag="mainB")
                        for ki, ko in enumerate(ko_order):
                            mmb = nc.tensor.matmul(
                                ps[:], ws[:, ko, :], u[ko][:, sl],
                                start=(ki == 0), stop=(ki == QLR_T - 1))
                            if m == NH and half == 0 and ki == 0:
                                gate_qb = mmb
                        cp = nc.scalar.copy(qT[m][:, sl], ps[:])
                        nc.vector.tensor_mul(qT[m][:, sl], qT[m][:, sl],
                                             qbc[half][:])
                    # rope transform for tile 8+j deferred until nope strip j:
                    # its DVE chain then overlaps a full strip of matmuls
                    if 4 <= mi < 8:
                        rope_xform(NH + mi - 4)
        es_qbc.close()
        es_u.close()

        # ======================= attention =======================
        es_avT = ExitStack()
        pavT = es_avT.enter_context(tc.tile_pool(name="avT", bufs=1))
        avT = [pavT.tile([128, S], BF, name=f"avT{i}") for i in range(NH)]
        with (
            tc.tile_pool(name="pbuf", bufs=4) as ppbuf,
            tc.tile_pool(name="zbuf", bufs=2) as pzbuf,
            # ppsc opened last: its banks free first at pool close, so
            # o_proj's psums land on banks h7 is already done with
            tc.tile_pool(name="ppav", bufs=2, space="PSUM") as ppav,
            tc.tile_pool(name="ppz", bufs=2, space="PSUM") as ppz,
            tc.tile_pool(name="ppsc", bufs=4, space="PSUM") as ppsc,
        ):
            for h in range(NH):
                with nc.named_scope(f"attn_h{h}"):
                    amul = attention_head(
                        tc, h, qT=qT, kpe2=kpe2, kn=kn_all[h],
                        v_sb=v_sb, avT=avT,
                        mask_sb=mask_sb, ones_s=ones_s,
                        ppbuf=ppbuf, pzbuf=pzbuf,
                        ppsc=ppsc, ppav=ppav, ppz=ppz)
                    if h == 0:
                        gate_attn = amul

        # ======================= o_proj =======================
        es_qT.close()
        with (
            tc.tile_pool(name="wo", bufs=2) as pwo,
            tc.tile_pool(name="osb", bufs=3) as posb,
            tc.tile_pool(name="ppo", bufs=6, space="PSUM") as ppo,
        ):
            with nc.named_scope("o_proj"):
                for hc in range(4):
                    ws = pwo.tile([128, NH, 512], BF, name=f"wo_s{hc}",
                                  tag="wo")
                    dma = nc.sync.dma_start(ws[:], wo[hc])
                    add_dep_helper(dma.ins, gate_qb.ins, sync=True,
                                   reason="stage wo DMA after q_b starts")
                    for t in range(NTC):
                        ps = ppo.tile([128, 512], F32, name=f"pso{hc}_{t}",
                                      tag="o")
                        for kk in range(NH):
                            nc.tensor.matmul(
                                ps[:], avT[kk][:, t * 128:(t + 1) * 128],
                                ws[:, kk, :],
                                start=(kk == 0), stop=(kk == NH - 1))
                        ot = posb.tile([128, 512], BF, name=f"ot{hc}_{t}",
                                       tag="ot")
                        nc.scalar.copy(ot[:], ps[:])
                        nc.gpsimd.dma_start(
                            out[t * 128:(t + 1) * 128,
                                hc * 512:(hc + 1) * 512], ot[:])
        es_avT.close()
        es_qT.close()
        es_kn.close()
        es_v.close()


def attention_head(tc, h, *, qT, kpe2, kn, v_sb, avT, mask_sb,
                   ones_s, ppbuf, pzbuf, ppsc, ppav, ppz):
    nc = tc.nc

    # roped q_pe lives in qT[8 + h//2] rows [(h%2)*64 : (h%2)*64+64];
    # kpe2 has k_pe duplicated in both 64-row halves so stationary and
    # moving operands share a partition base.
    qpe_src = qT[NH + h // 2]
    base = (h % 2) * 64

    # Two query-block pairs, chunk loops interleaved for PE lookahead.
    # pair p covers qblocks (2p, 2p+1) at columns p*512..(p+1)*512.
    st = []
    for p in range(2):
        q0 = 2 * p
        nw = 2 * q0 + 2
        st.append(dict(
            q0=q0, nw=nw, nk=nw + 2,
            psl=slice(p * 512, (p + 1) * 512),
            nsl=slice(p * 512 + 256, p * 512 + 512),
            ps_av=ppav.tile([128, 512], F32, name=f"psav{h}_{p}", tag="av"),
            ps_z=ppz.tile([128, 512], F32, name=f"psz{h}_{p}", tag="z"),
        ))
    pend = []

    def chunk(p, kc):
        s = st[p]
        ksl = slice(kc * 128, (kc + 1) * 128)
        wide = kc < s["nw"]
        csl = s["psl"] if wide else s["nsl"]
        cn = 512 if wide else 256
        ps_s = ppsc.tile([128, 512], F32, name=f"pss{h}_{p}_{kc}", tag="sc")
        nc.tensor.matmul(ps_s[:, :cn], kn[:, ksl], qT[h][:, csl],
                         start=True, stop=False)
        nc.tensor.matmul(ps_s[:, :cn], kpe2[base:base + 64, ksl],
                         qpe_src[base:base + 64, csl],
                         start=False, stop=True)
        d = kc - 2 * s["q0"] if wide else kc - 2 * (s["q0"] + 1)
        if d >= 0:
            nc.vector.tensor_add(ps_s[:, 0:256], ps_s[:, 0:256],
                                 mask_sb[d][:])
        p_sb = ppbuf.tile([128, 512], BF, name=f"p{h}_{p}_{kc}", tag="p")
        nc.scalar.activation(p_sb[:, :cn], ps_s[:, :cn], AF.Exp, scale=SCALE)

        def avz():
            vt = v_sb[h // 4][kc][:, (h % 4) * 128:(h % 4 + 1) * 128]
            osl = slice(0, 512) if wide else slice(256, 512)
            nc.tensor.matmul(s["ps_av"][:, osl], vt, p_sb[:, :cn],
                             start=(kc == 0), stop=(kc == s["nk"] - 1),
                             skip_group_check=True)
            nc.tensor.matmul(s["ps_z"][:, osl], ones_s[:], p_sb[:, :cn],
                             start=(kc == 0), stop=(kc == s["nk"] - 1),
                             skip_group_check=True)
        return avz

    def flush():
        for fn in pend:
            fn()
        pend.clear()

    for kc in range(st[1]["nk"]):
        cur = []
        if kc < st[0]["nk"]:
            cur.append(chunk(0, kc))
        cur.append(chunk(1, kc))
        flush()
        pend.extend(cur)
    flush()

    mul0 = None
    for p in range(2):
        s = st[p]
        zr = pzbuf.tile([128, 512], F32, name=f"zr{h}_{p}", tag="zr")
        nc.vector.reciprocal_approx_fast(zr[:], s["ps_z"][:])
        mul = nc.vector.tensor_mul(avT[h][:, s["psl"]], s["ps_av"][:], zr[:])
        if mul0 is None:
            mul0 = mul
    return mul0


# ---------------------------------------------------------------------------
# Host-side prep
# ---------------------------------------------------------------------------

def prepare_inputs(inputs: dict) -> list[dict]:
    """Full problem inputs -> list of 8 per-core input maps (bf16 pre-laid)."""
    import ml_dtypes
    BF_NP = ml_dtypes.bfloat16

    x = np.asarray(inputs["x"], np.float32)
    wq_a = np.asarray(inputs["wq_a"], np.float32)
    w_qa_ln = np.asarray(inputs["w_qa_ln"], np.float32)
    wq_b = np.asarray(inputs["wq_b"], np.float32)
    wkv_a = np.asarray(inputs["wkv_a"], np.float32)
    w_kva_ln = np.asarray(inputs["w_kva_ln"], np.float32)
    wk_b = np.asarray(inputs["wk_b"], np.float32)
    wv_b = np.asarray(inputs["wv_b"], np.float32)
    wo_w = np.asarray(inputs["wo"], np.float32)
    rotary_sin = np.asarray(inputs["rotary_sin"], np.float32)
    rotary_cos = np.asarray(inputs["rotary_cos"], np.float32)

    def strips(wT, n_out_chunks, ko_chunks, w_per=128):
        # wT: [K, W] -> [n_out_chunks, 128, ko_chunks, w_per]
        K, W = wT.shape
        a = wT.reshape(ko_chunks, 128, n_out_chunks, w_per)
        return np.ascontiguousarray(a.transpose(2, 1, 0, 3)).astype(BF_NP)

    wqaT = wq_a.T                                             # [HID, QLR]
    wqa_pre = strips(wqaT, QLR_T, HID_T)                      # [12,128,16,128]

    kv_perm = (list(range(KVLR))
               + [KVLR + 2 * i for i in range(ROPE // 2)]
               + [KVLR + 2 * i + 1 for i in range(ROPE // 2)])
    wkvaT = wkv_a[kv_perm, :].T                               # [HID, 576]
    wkva_pre = strips(wkvaT[:, :KVLR], KVLR_T, HID_T)         # [4,128,16,128]
    wkvar_pre = np.ascontiguousarray(
        wkvaT[:, KVLR:].reshape(HID_T, 128, ROPE).transpose(1, 0, 2)
    ).astype(BF_NP)                                           # [128,16,64]

    wq_b_eff = wq_b * w_qa_ln[None, :]
    wk_b_eff = wk_b * w_kva_ln[None, :]
    wv_b_eff = wv_b * w_kva_ln[None, :]

    per_group = []
    for g in range(2):
        heads = range(g * NH, (g + 1) * NH)
        qperm = [hh * (NOPE + ROPE) + dd for hh in heads for dd in range(NOPE)]
        for hh in heads:
            qperm += [hh * (NOPE + ROPE) + NOPE + 2 * i for i in range(ROPE // 2)]
            qperm += [hh * (NOPE + ROPE) + NOPE + 2 * i + 1
                      for i in range(ROPE // 2)]
        wqbT = wq_b_eff[qperm, :].T                           # [QLR, 1536]
        wqb_pre = strips(wqbT, QF_T, QLR_T)                   # [12,128,12,128]
        cols = [hh * NOPE + dd for hh in heads for dd in range(NOPE)]
        wkbT = wk_b_eff[cols, :].T                            # [KVLR, 1024]
        wkb_pre = np.ascontiguousarray(
            wkbT.reshape(KVLR_T, 128, NH * NOPE)).astype(BF_NP)
        wvbT = wv_b_eff[cols, :].T
        wvb_pre = np.ascontiguousarray(
            wvbT.reshape(KVLR_T, 128, NH * VD)).astype(BF_NP)
        woT = wo_w[:, cols].T                                 # [1024, HID]
        wo_pre = np.zeros((4, 128, NH, 512), np.float32)
        for hc in range(4):
            blk = woT[:, hc * 512:(hc + 1) * 512]             # [1024, 512]
            wo_pre[hc] = blk.reshape(NH, 128, 512).transpose(1, 0, 2)
        wo_pre = wo_pre.astype(BF_NP)
        per_group.append((wqb_pre, wkb_pre, wvb_pre, wo_pre))

    cosT = np.ascontiguousarray(rotary_cos.T)                 # [32, S]
    sinT = np.ascontiguousarray(rotary_sin.T)
    c128 = np.tile(cosT, (4, 1)).astype(BF_NP)                # [128, S]
    s128 = np.tile(sinT, (4, 1)).astype(BF_NP)

    pswm = np.zeros((128, 128), np.float32)
    for blk in (0, 64):
        for i in range(32):
            pswm[blk + i, blk + 32 + i] = 1.0     # t_b even-row -> odd out
            pswm[blk + 32 + i, blk + i] = -1.0    # t_b odd-row  -> even out
    pswm = pswm.astype(BF_NP)
    id128 = np.eye(128, dtype=np.float32).astype(BF_NP)

    kq = np.arange(128)[:, None]
    qq = np.arange(QB)[None, :]
    masks = np.stack([
        np.where(kq <= qq, 0.0, NEG).astype(np.float32),
        np.where(kq + 128 <= qq, 0.0, NEG).astype(np.float32),
    ])
    ones_col = np.ones((128, 1), np.float32).astype(BF_NP)
    ones_row = np.ones((1, 128), np.float32).astype(BF_NP)
    ones_sq = np.ones((128, 128), np.float32).astype(BF_NP)

    # x per sequence -> [128, 16, 1024] (p, ko, t)
    xp_b = []
    for b in range(B):
        xb = x[b * S:(b + 1) * S].T                           # [HID, S]
        xp_b.append(np.ascontiguousarray(
            xb.reshape(HID_T, 128, S).transpose(1, 0, 2)).astype(BF_NP))

    in_maps = []
    for c in range(8):
        b, g = c // 2, c % 2
        wqb_pre, wkb_pre, wvb_pre, wo_pre = per_group[g]
        in_maps.append(dict(
            xp=xp_b[b],
            wqa=np.ascontiguousarray(wqa_pre[g * QLOC:(g + 1) * QLOC]),
            wqb=wqb_pre,
            wkva=np.ascontiguousarray(wkva_pre[g * KLOC:(g + 1) * KLOC]),
            wkvar=wkvar_pre, wkb=wkb_pre, wvb=wvb_pre, wo=wo_pre,
            c128=c128, s128=s128, psw=pswm, id128=id128, masks=masks,
            ones_col=ones_col, ones_row=ones_row, ones_sq=ones_sq))
    return in_maps


def assemble_output(results: list[dict]) -> np.ndarray:
    outs = []
    for b in range(B):
        outs.append(results[2 * b]["out"].astype(np.float32)
                    + results[2 * b + 1]["out"].astype(np.float32))
    return np.concatenate(outs, axis=0)


# ---------------------------------------------------------------------------
# Harness entry point: full inputs in, full output out.
# ---------------------------------------------------------------------------

_NC_CACHE = []


def _get_nc():
    if not _NC_CACHE:
        _NC_CACHE.append(build_nc())
    return _NC_CACHE[0]


def kernel(_profile=False, **inputs) -> np.ndarray:
    """MLA attention on 8 NeuronCores: 4-way data-parallel over sequences x
    2-way tensor-parallel over heads. Takes full (unsharded) inputs, returns
    the full [4096, 2048] float32 output."""
    from concourse.bass_utils import run_bass_kernel_spmd

    seqstarts = np.asarray(inputs["seqstarts"])
    b = seqstarts.shape[0] - 1
    assert b == B and np.all(np.diff(seqstarts) == S), (
        "kernel compiled for 4 uniform sequences of 1024 tokens")

    nc = _get_nc()
    in_maps = prepare_inputs(inputs)
    kwargs = {}
    if _profile:
        _install_ntff_hook()
        kwargs = dict(trace=True, trace_cores=list(range(8)))
    res = run_bass_kernel_spmd(nc, in_maps, list(range(8)), **kwargs)
    out = assemble_output(res.results).astype(np.float32)
    if _profile:
        return out, res
    return out


def _install_ntff_hook():
    """The agent image lacks antenv.axon_hooks; reconstruct the NTFF profile
    hook via ctypes so run_bass_kernel_spmd(trace=True) works (profiling-only
    path, used by test.py)."""
    import types
    if 'antenv.axon_hooks' in sys.modules:
        return
    try:
        from trn_agent_boot.trn_boot import _ntff_profile_via_ctypes
        hook = _ntff_profile_via_ctypes('/opt/axon/libaxon_pjrt.so')
    except Exception:
        hook = None
    mod = types.ModuleType('antenv.axon_hooks')
    mod.get_axon_ntff_profile_hook = lambda: hook
    sys.modules['antenv.axon_hooks'] = mod


# revision 59
# speedup vs baseline: 1.0963x; 1.0963x over previous
"""MLA attention TRN2 kernel: 4-way data-parallel (sequences) x 2-way
tensor-parallel (heads). Each core: 1 sequence (1024 tokens), 8 heads.

v2: all matmuls in bf16 (fp32r runs in fp32-HIGH PE mode at ~1.3 cyc/row
with slow non-FWL weight loads; bf16 gets 1.0 cyc/row + fast weight
load). RoPE done as batched full-width DVE muls + PE signed-swap
matmuls instead of 32-row DVE ops. A warmup matmul stream at program
start flips the PE HAM clock gate to 2.4 GHz while the x DMA streams.
Weights are pre-laid out on host into SBUF tile shapes so all weight
DMAs are contiguous.

Layout convention: features on partitions, tokens on free axis; scores
computed transposed [k, q] so softmax sums use PE ones-matmuls and no
transposes are needed anywhere. Softmax denominator folded into avT via
per-(head, qblock-pair) broadcast multiply.
"""
import sys
sys.path.insert(0, '/opt/trn_rl_repo')

import math
import numpy as np

import concourse.bass as bass
import concourse.tile as tile
from concourse.tile_rust import add_dep_helper
from concourse import bacc, mybir

BF = mybir.dt.bfloat16
F32 = mybir.dt.float32
AF = mybir.ActivationFunctionType

H = 16
NH = 8            # heads per core
NOPE = 128
ROPE = 64
VD = 128
HID = 2048
QLR = 1536
KVLR = 512
B = 4
S = 1024
EPS = 1e-6
SCALE = 1.0 / math.sqrt(NOPE + ROPE)
NEG = -1.0e30

HID_T = HID // 128    # 16
QLR_T = QLR // 128    # 12
KVLR_T = KVLR // 128  # 4
QLOC = QLR_T // 2     # 6 u strips computed locally (feature-split pair)
KLOC = KVLR_T // 2    # 2 ckv strips computed locally
CC_GROUPS = [[0, 1], [2, 3], [4, 5], [6, 7]]
QF = NH * (NOPE + ROPE)   # 1536 q features per core
QF_T = QF // 128          # 12 (chunks 0..7 nope, 8..11 rope)
NTC = S // 128            # 8 token chunks
QB = 256                  # query block
NQB = S // QB             # 4
N_WARM = 24               # warmup matmuls (keep HAM busy >3.4us)


def build_nc():
    nc = bacc.Bacc("TRN2", target_bir_lowering=False, debug=False, num_devices=8)

    xp = nc.dram_tensor("xp", [128, HID_T, S], BF, kind="ExternalInput")
    wqa = nc.dram_tensor("wqa", [QLOC, 128, HID_T, 128], BF, kind="ExternalInput")
    wqb = nc.dram_tensor("wqb", [QF_T, 128, QLR_T, 128], BF, kind="ExternalInput")
    wkva = nc.dram_tensor("wkva", [KLOC, 128, HID_T, 128], BF, kind="ExternalInput")
    wkvar = nc.dram_tensor("wkvar", [128, HID_T, ROPE], BF, kind="ExternalInput")
    wkb = nc.dram_tensor("wkb", [KVLR_T, 128, NH * NOPE], BF, kind="ExternalInput")
    wvb = nc.dram_tensor("wvb", [KVLR_T, 128, NH * VD], BF, kind="ExternalInput")
    wo = nc.dram_tensor("wo", [4, 128, NH, 512], BF, kind="ExternalInput")
    c128 = nc.dram_tensor("c128", [128, S], BF, kind="ExternalInput")
    s128 = nc.dram_tensor("s128", [128, S], BF, kind="ExternalInput")
    psw = nc.dram_tensor("psw", [128, 128], BF, kind="ExternalInput")
    id128 = nc.dram_tensor("id128", [128, 128], BF, kind="ExternalInput")
    masks = nc.dram_tensor("masks", [2, 128, QB], F32, kind="ExternalInput")
    ones_col = nc.dram_tensor("ones_col", [128, 1], BF, kind="ExternalInput")
    ones_row = nc.dram_tensor("ones_row", [1, 128], BF, kind="ExternalInput")
    ones_sq = nc.dram_tensor("ones_sq", [128, 128], BF, kind="ExternalInput")
    out = nc.dram_tensor("out", [S, HID], BF, kind="ExternalOutput")
    # pair-exchange scratch: u strips streamed as 3 gathers of 2 strips each
    # (flat row-major; last one carries the z2q partial row), ckv as one.
    du_loc = [nc.dram_tensor(f"du_loc{i}", [256, S],
                             BF, kind="Internal") for i in range(3)]
    du_gat = [nc.dram_tensor(f"du_gat{i}", [2, 256, S],
                             BF, kind="Internal") for i in range(3)]
    dz_loc = nc.dram_tensor("dz_loc", [1, S], BF, kind="Internal")
    dz_gat = nc.dram_tensor("dz_gat", [2, 1, S], BF, kind="Internal")
    dk_loc = nc.dram_tensor("dk_loc", [KLOC * 128 + 1, S], BF, kind="Internal")
    dk_gat = nc.dram_tensor("dk_gat", [2, KLOC * 128 + 1, S], BF, kind="Internal")

    with tile.TileContext(nc, pool_alloc_mode="queue") as tc:
        build_body(tc, xp=xp, wqa=wqa, wqb=wqb, wkva=wkva, wkvar=wkvar,
                   wkb=wkb, wvb=wvb, wo=wo, c128=c128, s128=s128, psw=psw,
                   id128=id128, masks=masks, ones_col=ones_col,
                   ones_row=ones_row, ones_sq=ones_sq, out=out,
                   du_loc=du_loc, du_gat=du_gat, dk_loc=dk_loc, dk_gat=dk_gat,
                   dz_loc=dz_loc, dz_gat=dz_gat)
    nc.compile()
    return nc


def build_body(tc, *, xp, wqa, wqb, wkva, wkvar, wkb, wvb, wo, c128, s128,
               psw, id128, masks, ones_col, ones_row, ones_sq, out,
               du_loc, du_gat, dk_loc, dk_gat, dz_loc, dz_gat):
    from contextlib import ExitStack
    nc = tc.nc

    with (
        tc.tile_pool(name="const", bufs=1) as pconst,
        tc.tile_pool(name="ckv", bufs=1) as pckv,
    ):
        ones_c = pconst.tile([128, 1], BF, name="ones_c")
        nc.sync.dma_start(ones_c[:], ones_col[:])
        ones_r = pconst.tile([1, 128], BF, name="ones_r")
        nc.sync.dma_start(ones_r[:], ones_row[:])
        ones_s = pconst.tile([128, 128], BF, name="ones_s")
        nc.sync.dma_start(ones_s[:], ones_sq[:])
        cos_sb = pconst.tile([128, S], BF, name="cos_sb")
        nc.sync.dma_start(cos_sb[:], c128[:])
        sin_sb = pconst.tile([128, S], BF, name="sin_sb")
        nc.sync.dma_start(sin_sb[:], s128[:])
        psw_sb = pconst.tile([128, 128], BF, name="psw_sb")
        nc.sync.dma_start(psw_sb[:], psw[:])
        id_sb = pconst.tile([128, 128], BF, name="id_sb")
        nc.sync.dma_start(id_sb[:], id128[:])
        mask_sb = []
        for i in range(2):
            m = pconst.tile([128, QB], F32, name=f"mask{i}")
            nc.sync.dma_start(m[:], masks[i])
            mask_sb.append(m)
        kpe2 = pconst.tile([128, S], BF, name="kpe2")
        eps_t = pconst.tile([1, 1], F32, name="eps_t")
        nc.vector.memset(eps_t[:], EPS)

        ckv = [pckv.tile([128, S], BF, name=f"ckv{i}") for i in range(KVLR_T)]

        # ---- HAM warmup: keep PE busy while x/wqa DMAs stream ----
        with tc.tile_pool(name="pwarm", bufs=1, space="PSUM") as ppw:
            wps = ppw.tile([128, 512], F32, name="warm_ps")
            for i in range(N_WARM):
                nc.tensor.matmul(wps[:], ones_s[:], cos_sb[:, 0:512],
                                 start=True, stop=True, skip_group_check=True)

        # ============ Phases: q_a -> kv_a (norms overlap) -> q_b ============
        es_u = ExitStack()
        pu = es_u.enter_context(tc.tile_pool(name="u", bufs=1))
        es_qbc = ExitStack()
        pqbc = es_qbc.enter_context(tc.tile_pool(name="qbc", bufs=1))
        qbc = [pqbc.tile([128, 512], BF, name=f"qbcn{i}") for i in range(2)]
        es_x = ExitStack()
        px = es_x.enter_context(tc.tile_pool(name="xsb", bufs=1))

        xsb = px.tile([128, HID_T, S], BF, name="xsb")
        prev_x = {0: None, 1: None}
        for kb in range(4):
            eng = nc.gpsimd if kb % 2 == 0 else nc.sync
            dma = eng.dma_start(xsb[:, kb * 4:(kb + 1) * 4, :],
                                xp[:, kb * 4:(kb + 1) * 4, :])
            if prev_x[kb % 2] is not None:
                add_dep_helper(dma.ins, prev_x[kb % 2].ins, sync=True,
                               reason="serialize x waves so early tiles land first")
            prev_x[kb % 2] = dma
        u = [pu.tile([128, S], BF, name=f"u{i}") for i in range(QLR_T)]

        with (
            tc.tile_pool(name="wstripA", bufs=4) as pwA,
            tc.tile_pool(name="sqA", bufs=4) as psq,
            tc.tile_pool(name="normA", bufs=2) as pnorm,
            tc.tile_pool(name="ppmain", bufs=4, space="PSUM") as ppm,
            tc.tile_pool(name="ppz2", bufs=2, space="PSUM") as ppz2,
            tc.tile_pool(name="ppbc", bufs=2, space="PSUM") as ppbc,
        ):
            # ---- kv_a first: local 2 ckv strips + k_pe strip (both cores);
            # its small gather then hides under q_a ----
            z2k = [ppz2.tile([1, 512], F32, name=f"z2k{h}", tag="z2")
                   for h in range(2)]
            kv4 = psq.tile([64, S], BF, name="kv4", tag="kv4", bufs=1)
            ckstage = [pu.tile([128, S], BF, name=f"ckst{i}")
                       for i in range(KLOC)]
            zkstage = pnorm.tile([1, S], BF, name="zkstage", tag="zst", bufs=1)
            k_dmas = []
            pend = []
            with nc.named_scope("kv_a"):
                for m in range(KLOC):
                    ws = pwA.tile([128, HID_T, 128], BF, name=f"wkva_s{m}",
                                  tag="wstrip")
                    nc.sync.dma_start(ws[:], wkva[m])
                    for half in range(2):
                        sl = slice(half * 512, (half + 1) * 512)
                        ps = ppm.tile([128, 512], F32, name=f"pskv{m}_{half}",
                                      tag="main")
                        for ko in range(HID_T):
                            mmk = nc.tensor.matmul(
                                ps[:], ws[:, ko, :], xsb[:, ko, sl],
                                start=(ko == 0), stop=(ko == HID_T - 1))
                            if m == 0 and half == 0 and ko == 0:
                                gate_kva = mmk
                        cpk = nc.scalar.copy(ckstage[m][:, sl], ps[:])
                        sq = psq.tile([128, 512], BF, name=f"sqk{m}_{half}",
                                      tag="sq")
                        nc.scalar.activation(sq[:], ps[:], AF.Square)
                        for fn in pend:
                            fn()
                        pend = []
                        pend.append(
                            lambda sq=sq, half=half, m=m: nc.tensor.matmul(
                                z2k[half][:], ones_c[:], sq[:],
                                start=(m == 0), stop=(m == KLOC - 1),
                                skip_group_check=True))
                    k_dmas.append(nc.gpsimd.dma_start(
                        dk_loc[m * 128:(m + 1) * 128, :], ckstage[m][:]))
                for fn in pend:
                    fn()
                pend = []
                for half in range(2):
                    sl = slice(half * 512, (half + 1) * 512)
                    nc.scalar.copy(zkstage[:, sl], z2k[half][:])
                k_dmas.append(
                    nc.gpsimd.dma_start(dk_loc[KLOC * 128:, :], zkstage[:]))
                cck = nc.gpsimd.collective_compute(
                    "AllGather", mybir.AluOpType.bypass,
                    replica_groups=CC_GROUPS,
                    ins=[dk_loc[:]], outs=[dk_gat[:]])
                for dma in k_dmas:
                    add_dep_helper(cck.ins, dma.ins, sync=True,
                                   reason="gather after ckv stage DMAs")
                for ko in range(KVLR_T):
                    g, i = ko // KLOC, ko % KLOC
                    dma = nc.gpsimd.dma_start(
                        ckv[ko][:], dk_gat[g, i * 128:(i + 1) * 128, :])
                    add_dep_helper(dma.ins, cck.ins, sync=True,
                                   reason="read ckv after gather")
                zkg = [pnorm.tile([1, S], BF, name=f"zkg{g}", tag=f"zkg{g}",
                                  bufs=1) for g in range(2)]
                for g in range(2):
                    dma = nc.gpsimd.dma_start(zkg[g][:],
                                              dk_gat[g, KLOC * 128:, :])
                    add_dep_helper(dma.ins, cck.ins, sync=True,
                                   reason="read z2k after gather")
                zkt = pnorm.tile([1, S], F32, name="zkt", tag="zkt", bufs=1)
                nc.vector.tensor_add(zkt[:], zkg[0][:], zkg[1][:])

            # ---- q_a: local 6 u strips; streamed pairwise AllGathers ----
            z2q = [ppz2.tile([1, 512], F32, name=f"z2q{h}", tag="z2")
                   for h in range(2)]
            ustage = [pu.tile([128, S], BF, name=f"ust{i}")
                      for i in range(QLOC)]
            zqstage = pnorm.tile([1, S], BF, name="zqstage", tag="zst", bufs=1)
            u_dmas = []
            zqg = [pqbc.tile([1, S], BF, name=f"zqg{g}") for g in range(2)]
            zqt = pqbc.tile([1, S], F32, name="zqt")

            def u_gather(cc_idx):
                # gather strips (2*cc_idx, 2*cc_idx+1)
                cc = nc.gpsimd.collective_compute(
                    "AllGather", mybir.AluOpType.bypass,
                    replica_groups=CC_GROUPS,
                    ins=[du_loc[cc_idx][:]], outs=[du_gat[cc_idx][:]])
                for dma in u_dmas:
                    add_dep_helper(cc.ins, dma.ins, sync=True,
                                   reason="gather after u stage DMAs")
                u_dmas.clear()
                for g in range(2):
                    for i in range(2):
                        ko = g * QLOC + cc_idx * 2 + i
                        dma = nc.gpsimd.dma_start(
                            u[ko][:],
                            du_gat[cc_idx][g, i * 128:(i + 1) * 128, :])
                        add_dep_helper(dma.ins, cc.ins, sync=True,
                                       reason="read u after gather")

            with nc.named_scope("q_a"):
                for m in range(QLOC):
                    ws = pwA.tile([128, HID_T, 128], BF, name=f"wqa_s{m}",
                                  tag="wstrip")
                    dma = nc.sync.dma_start(ws[:], wqa[m])
                    add_dep_helper(dma.ins, gate_kva.ins, sync=True,
                                   reason="stage wqa DMA after kv_a starts")
                    for half in range(2):
                        sl = slice(half * 512, (half + 1) * 512)
                        ps = ppm.tile([128, 512], F32, name=f"psu{m}_{half}",
                                      tag="main")
                        for ko in range(HID_T):
                            mmq = nc.tensor.matmul(
                                ps[:], ws[:, ko, :], xsb[:, ko, sl],
                                start=(ko == 0), stop=(ko == HID_T - 1))
                            if m == 0 and half == 0 and ko == 0:
                                gate_qa = mmq
                        cp = nc.scalar.copy(ustage[m][:, sl], ps[:])
                        sq = psq.tile([128, 512], BF, name=f"squ{m}_{half}",
                                      tag="sq")
                        nc.scalar.activation(sq[:], ps[:], AF.Square)
                        for fn in pend:
                            fn()
                        pend = []
                        pend.append(
                            lambda sq=sq, half=half, m=m: nc.tensor.matmul(
                                z2q[half][:], ones_c[:], sq[:],
                                start=(m == 0), stop=(m == QLOC - 1),
                                skip_group_check=True))
                    u_dmas.append(nc.gpsimd.dma_start(
                        du_loc[m // 2][(m % 2) * 128:(m % 2 + 1) * 128, :],
                        ustage[m][:]))
                    if m in (1, 3, 5):
                        u_gather(m // 2)
                for fn in pend:
                    fn()
                pend = []
                # z2q partials ride their own tiny gather so the u strips
                # aren't held back behind the z2q psum stop
                for half in range(2):
                    sl = slice(half * 512, (half + 1) * 512)
                    nc.scalar.copy(zqstage[:, sl], z2q[half][:])
                zdma = nc.gpsimd.dma_start(dz_loc[:], zqstage[:])
                ccz = nc.gpsimd.collective_compute(
                    "AllGather", mybir.AluOpType.bypass,
                    replica_groups=CC_GROUPS,
                    ins=[dz_loc[:]], outs=[dz_gat[:]])
                add_dep_helper(ccz.ins, zdma.ins, sync=True,
                               reason="z gather after stage")
                for g in range(2):
                    dma = nc.gpsimd.dma_start(zqg[g][:], dz_gat[g])
                    add_dep_helper(dma.ins, ccz.ins, sync=True,
                                   reason="read z2q after gather")
                nc.vector.tensor_add(zqt[:], zqg[0][:], zqg[1][:])

            # ---- k_pe strip (independent of the gathers: fills the
            # cck latency window right after q_a) ----
            with nc.named_scope("kv_rope"):
                ws = pwA.tile([128, HID_T, ROPE], BF, name="wkva_s4",
                              tag="wstrip")
                nc.sync.dma_start(ws[:], wkvar[:])
                for half in range(2):
                    sl = slice(half * 512, (half + 1) * 512)
                    ps = ppm.tile([128, 512], F32, name=f"pskvr_{half}",
                                  tag="main")
                    for ko in range(HID_T):
                        nc.tensor.matmul(
                            ps[:ROPE, :], ws[:, ko, :], xsb[:, ko, sl],
                            start=(ko == 0), stop=(ko == HID_T - 1))
                    nc.scalar.copy(kv4[:, sl], ps[:ROPE, :])

            # ---- ckv rmsnorm scale + k_pe RoPE (PE signed-swap) ----
            with nc.named_scope("ckv_norm"):
                for half in range(2):
                    sl = slice(half * 512, (half + 1) * 512)
                    rt = pnorm.tile([1, 512], F32, name=f"krt{half}", tag="rt", bufs=1)
                    nc.scalar.activation(rt[:], zkt[:, sl], AF.Sqrt,
                                         scale=1.0 / KVLR, bias=eps_t[:])
                    rr = pnorm.tile([1, 512], BF, name=f"krr{half}", tag="rr", bufs=1)
                    with nc.allow_low_precision(reason="bf16 rms factor"):
                        nc.vector.reciprocal(rr[:], rt[:])
                    bc = pnorm.tile([128, 512], BF, name=f"kbc{half}", tag="bc")
                    nc.gpsimd.partition_broadcast(bc[:], rr[:])
                    for m in range(KVLR_T):
                        nc.vector.tensor_mul(ckv[m][:, sl], ckv[m][:, sl],
                                             bc[:])

                # k_pe RoPE: elementwise cos/sin muls then PE pair-swap
                kta = pnorm.tile([64, S], BF, name="kta", tag="r64a", bufs=1)
                nc.vector.tensor_mul(kta[:], kv4[:], cos_sb[0:64, :])
                ktb = pnorm.tile([64, S], BF, name="ktb", tag="r64b", bufs=1)
                nc.vector.tensor_mul(ktb[:], kv4[:], sin_sb[0:64, :])
                for half in range(2):
                    sl = slice(half * 512, (half + 1) * 512)
                    pk = ppbc.tile([64, 512], F32, name=f"pkpe{half}", tag="bc")
                    nc.tensor.matmul(pk[:], psw_sb[0:64, 0:64], ktb[:, sl],
                                     start=True, stop=False)
                    nc.tensor.matmul(pk[:], id_sb[0:64, 0:64], kta[:, sl],
                                     start=False, stop=True)
                    nc.scalar.copy(kpe2[0:64, sl], pk[:])
                nc.gpsimd.dma_start(kpe2[64:128, :], kpe2[0:64, :])

        es_x.close()

        # ---- v_b early: covers the tail of the u AllGather ----
        es_v = ExitStack()
        pv = es_v.enter_context(tc.tile_pool(name="v", bufs=1, side="right"))
        v_sb = [[pv.tile([128, 512], BF, name=f"v{g}_{t}")
                 for t in range(NTC)] for g in range(2)]
        with (
            tc.tile_pool(name="wvb", bufs=1) as pwvb,
            tc.tile_pool(name="ppv", bufs=4, space="PSUM") as ppv,
        ):
            wvb_sb = [pwvb.tile([128, NH * VD], BF, name=f"wvb{i}")
                      for i in range(KVLR_T)]
            for ko in range(KVLR_T):
                dma = nc.sync.dma_start(wvb_sb[ko][:], wvb[ko])
                add_dep_helper(dma.ins, gate_qa.ins, sync=True,
                               reason="stage wvb DMA after q_a starts")
            with nc.named_scope("v_b"):
                for g in range(2):
                    for t in range(NTC):
                        ps = ppv.tile([128, 512], F32,
                                      name=f"psv{g}_{t}", tag="v")
                        for kk in range(KVLR_T):
                            nc.tensor.matmul(
                                ps[:],
                                ckv[kk][:, t * 128:(t + 1) * 128],
                                wvb_sb[kk][:, g * 512:(g + 1) * 512],
                                start=(kk == 0),
                                stop=(kk == KVLR_T - 1))
                        nc.scalar.copy(v_sb[g][t][:], ps[:])

        # ---- k_nope for all heads: fills the u-AllGather latency window ----
        es_kn = ExitStack()
        pkn = es_kn.enter_context(tc.tile_pool(name="kn", bufs=1, side="right"))
        kn_all = [pkn.tile([128, S], BF, name=f"kn{h}") for h in range(NH)]
        es_wkb = ExitStack()
        pwkb = es_wkb.enter_context(tc.tile_pool(name="wkb", bufs=1))
        wkb_sb = [pwkb.tile([128, NH * NOPE], BF, name=f"wkb{i}")
                  for i in range(KVLR_T)]
        for ko in range(KVLR_T):
            dma = nc.sync.dma_start(wkb_sb[ko][:], wkb[ko])
            add_dep_helper(dma.ins, gate_qa.ins, sync=True,
                           reason="stage wkb DMA after q_a starts")
        with tc.tile_pool(name="ppkn", bufs=2, space="PSUM") as ppkn:
            with nc.named_scope("k_b"):
                for hh in range(NH):
                    for half in range(2):
                        sl = slice(half * 512, (half + 1) * 512)
                        ps = ppkn.tile([128, 512], F32,
                                       name=f"pskn{hh}_{half}", tag="kn")
                        for kk in range(KVLR_T):
                            nc.tensor.matmul(
                                ps[:], wkb_sb[kk][:, hh * 128:(hh + 1) * 128],
                                ckv[kk][:, sl],
                                start=(kk == 0), stop=(kk == KVLR_T - 1))
                        nc.scalar.copy(kn_all[hh][:, sl], ps[:])
        es_wkb.close()

        # ---- u rmsnorm factor: broadcast on gpsimd so no PE op ever
        # blocks on the z2q gather chain ----
        with tc.tile_pool(name="normB", bufs=1) as pn2:
            with nc.named_scope("u_norm"):
                for half in range(2):
                    sl = slice(half * 512, (half + 1) * 512)
                    rt = pn2.tile([1, 512], F32, name=f"qrt{half}")
                    nc.scalar.activation(rt[:], zqt[:, sl], AF.Sqrt,
                                         scale=1.0 / QLR, bias=eps_t[:])
                    rr = pn2.tile([1, 512], BF, name=f"qrr{half}")
                    with nc.allow_low_precision(reason="bf16 rms factor"):
                        nc.vector.reciprocal(rr[:], rt[:])
                    nc.gpsimd.partition_broadcast(qbc[half][:], rr[:])

        # ---- q_b: qT = wqbT.T @ u -> [1536, S]; rope tiles first ----
        es_qT = ExitStack()
        pqT = es_qT.enter_context(tc.tile_pool(name="qT", bufs=1, side="right"))
        qT = [pqT.tile([128, S], BF, name=f"qTt{i}") for i in range(QF_T)]
        with (
            tc.tile_pool(name="wstripB", bufs=4) as pwB,
            tc.tile_pool(name="ropeB", bufs=2) as prope,
            tc.tile_pool(name="ppmainB", bufs=6, space="PSUM") as ppmB,
            tc.tile_pool(name="pprope", bufs=2, space="PSUM") as ppr,
        ):
            def rope_xform(m):
                # rope transform in place: qT[m] <- ta + Psw @ tb
                ta = prope.tile([128, S], BF, name=f"rta{m}", tag="ta")
                nc.vector.tensor_mul(ta[:], qT[m][:], cos_sb[:])
                tb = prope.tile([128, S], BF, name=f"rtb{m}", tag="tb")
                nc.vector.tensor_mul(tb[:], qT[m][:], sin_sb[:])
                for half in range(2):
                    sl = slice(half * 512, (half + 1) * 512)
                    pr = ppr.tile([128, 512], F32, name=f"psr{m}_{half}",
                                  tag="rope")
                    nc.tensor.matmul(pr[:], psw_sb[:], tb[:, sl],
                                     start=True, stop=False)
                    nc.tensor.matmul(pr[:], id_sb[:], ta[:, sl],
                                     start=False, stop=True)
                    nc.scalar.copy(qT[m][:, sl], pr[:])

            with nc.named_scope("q_b"):
                for mi, m in enumerate(list(range(NH, QF_T)) + list(range(NH))):
                    ws = pwB.tile([128, QLR_T, 128], BF, name=f"wqb_s{m}",
                                  tag="wstripB")
                    dma = nc.sync.dma_start(ws[:], wqb[m])
                    add_dep_helper(dma.ins, gate_kva.ins, sync=True,
                                   reason="stage wqb DMA after kv_a starts")
                    # contract over ko with the last-gathered strips at the
                    # end, so early q_b groups don't wait on the final CC
                    ko_order = [0, 1, 2, 3, 6, 7, 8, 9, 4, 5, 10, 11]
                    for half in range(2):
                        sl = slice(half * 512, (half + 1) * 512)
                        ps = ppmB.tile([128, 512], F32, name=f"psq{m}_{half}",
                                       tag="mainB")
                        for ki, ko in enumerate(ko_order):
                            mmb = nc.tensor.matmul(
                                ps[:], ws[:, ko, :], u[ko][:, sl],
                                start=(ki == 0), stop=(ki == QLR_T - 1))
                            if m == NH and half == 0 and ki == 0:
                                gate_qb = mmb
                        cp = nc.scalar.copy(qT[m][:, sl], ps[:])
                        nc.vector.tensor_mul(qT[m][:, sl], qT[m][:, sl],
                                             qbc[half][:])
                    # rope transform for tile 8+j deferred until nope strip j:
                    # its DVE chain then overlaps a full strip of matmuls
                    if 4 <= mi < 8:
                        rope_xform(NH + mi - 4)
        es_qbc.close()
        es_u.close()

        # ======================= attention =======================
        es_avT = ExitStack()
        pavT = es_avT.enter_context(tc.tile_pool(name="avT", bufs=1))
        avT = [pavT.tile([128, S], BF, name=f"avT{i}") for i in range(NH)]
        with (
            tc.tile_pool(name="pbuf", bufs=4) as ppbuf,
            tc.tile_pool(name="zbuf", bufs=2) as pzbuf,
            # ppsc opened last: its banks free first at pool close, so
            # o_proj's psums land on banks h7 is already done with
            tc.tile_pool(name="ppav", bufs=2, space="PSUM") as ppav,
            tc.tile_pool(name="ppz", bufs=2, space="PSUM") as ppz,
            tc.tile_pool(name="ppsc", bufs=4, space="PSUM") as ppsc,
        ):
            for h in range(NH):
                with nc.named_scope(f"attn_h{h}"):
                    amul = attention_head(
                        tc, h, qT=qT, kpe2=kpe2, kn=kn_all[h],
                        v_sb=v_sb, avT=avT,
                        mask_sb=mask_sb, ones_s=ones_s,
                        ppbuf=ppbuf, pzbuf=pzbuf,
                        ppsc=ppsc, ppav=ppav, ppz=ppz)
                    if h == 0:
                        gate_attn = amul

        # ======================= o_proj =======================
        es_qT.close()
        with (
            tc.tile_pool(name="wo", bufs=2) as pwo,
            tc.tile_pool(name="osb", bufs=3) as posb,
            tc.tile_pool(name="ppo", bufs=6, space="PSUM") as ppo,
        ):
            with nc.named_scope("o_proj"):
                for hc in range(4):
                    ws = pwo.tile([128, NH, 512], BF, name=f"wo_s{hc}",
                                  tag="wo")
                    dma = nc.sync.dma_start(ws[:], wo[hc])
                    add_dep_helper(dma.ins, gate_qb.ins, sync=True,
                                   reason="stage wo DMA after q_b starts")
                    for t in range(NTC):
                        ps = ppo.tile([128, 512], F32, name=f"pso{hc}_{t}",
                                      tag="o")
                        for kk in range(NH):
                            nc.tensor.matmul(
                                ps[:], avT[kk][:, t * 128:(t + 1) * 128],
                                ws[:, kk, :],
                                start=(kk == 0), stop=(kk == NH - 1))
                        ot = posb.tile([128, 512], BF, name=f"ot{hc}_{t}",
                                       tag="ot")
                        nc.scalar.copy(ot[:], ps[:])
                        nc.gpsimd.dma_start(
                            out[t * 128:(t + 1) * 128,
                                hc * 512:(hc + 1) * 512], ot[:])
        es_avT.close()
        es_qT.close()
        es_kn.close()
        es_v.close()


def attention_head(tc, h, *, qT, kpe2, kn, v_sb, avT, mask_sb,
                   ones_s, ppbuf, pzbuf, ppsc, ppav, ppz):
    nc = tc.nc

    # roped q_pe lives in qT[8 + h//2] rows [(h%2)*64 : (h%2)*64+64];
    # kpe2 has k_pe duplicated in both 64-row halves so stationary and
    # moving operands share a partition base.
    qpe_src = qT[NH + h // 2]
    base = (h % 2) * 64

    # Two query-block pairs, chunk loops interleaved for PE lookahead.
    # pair p covers qblocks (2p, 2p+1) at columns p*512..(p+1)*512.
    st = []
    for p in range(2):
        q0 = 2 * p
        nw = 2 * q0 + 2
        st.append(dict(
            q0=q0, nw=nw, nk=nw + 2,
            psl=slice(p * 512, (p + 1) * 512),
            nsl=slice(p * 512 + 256, p * 512 + 512),
            ps_av=ppav.tile([128, 512], F32, name=f"psav{h}_{p}", tag="av"),
            ps_z=ppz.tile([128, 512], F32, name=f"psz{h}_{p}", tag="z"),
        ))
    pend = []

    def chunk(p, kc):
        s = st[p]
        ksl = slice(kc * 128, (kc + 1) * 128)
        wide = kc < s["nw"]
        csl = s["psl"] if wide else s["nsl"]
        cn = 512 if wide else 256
        ps_s = ppsc.tile([128, 512], F32, name=f"pss{h}_{p}_{kc}", tag="sc")
        nc.tensor.matmul(ps_s[:, :cn], kn[:, ksl], qT[h][:, csl],
                         start=True, stop=False)
        nc.tensor.matmul(ps_s[:, :cn], kpe2[base:base + 64, ksl],
                         qpe_src[base:base + 64, csl],
                         start=False, stop=True)
        d = kc - 2 * s["q0"] if wide else kc - 2 * (s["q0"] + 1)
        if d >= 0:
            nc.vector.tensor_add(ps_s[:, 0:256], ps_s[:, 0:256],
                                 mask_sb[d][:])
        p_sb = ppbuf.tile([128, 512], BF, name=f"p{h}_{p}_{kc}", tag="p")
        nc.scalar.activation(p_sb[:, :cn], ps_s[:, :cn], AF.Exp, scale=SCALE)

        def avz():
            vt = v_sb[h // 4][kc][:, (h % 4) * 128:(h % 4 + 1) * 128]
            osl = slice(0, 512) if wide else slice(256, 512)
            nc.tensor.matmul(s["ps_av"][:, osl], vt, p_sb[:, :cn],
                             start=(kc == 0), stop=(kc == s["nk"] - 1),
                             skip_group_check=True)
            nc.tensor.matmul(s["ps_z"][:, osl], ones_s[:], p_sb[:, :cn],
                             start=(kc == 0), stop=(kc == s["nk"] - 1),
                             skip_group_check=True)
        return avz

    def flush():
        for fn in pend:
            fn()
        pend.clear()

    for kc in range(st[1]["nk"]):
        cur = []
        if kc < st[0]["nk"]:
            cur.append(chunk(0, kc))
        cur.append(chunk(1, kc))
        flush()
        pend.extend(cur)
    flush()

    mul0 = None
    for p in range(2):
        s = st[p]
        zr = pzbuf.tile([128, 512], F32, name=f"zr{h}_{p}", tag="zr")
        nc.vector.reciprocal_approx_fast(zr[:], s["ps_z"][:])
        mul = nc.vector.tensor_mul(avT[h][:, s["psl"]], s["ps_av"][:], zr[:])
        if mul0 is None:
            mul0 = mul
    return mul0


# ---------------------------------------------------------------------------
# Host-side prep
# ---------------------------------------------------------------------------

def prepare_inputs(inputs: dict) -> list[dict]:
    """Full problem inputs -> list of 8 per-core input maps (bf16 pre-laid)."""
    import ml_dtypes
    BF_NP = ml_dtypes.bfloat16

    x = np.asarray(inputs["x"], np.float32)
    wq_a = np.asarray(inputs["wq_a"], np.float32)
    w_qa_ln = np.asarray(inputs["w_qa_ln"], np.float32)
    wq_b = np.asarray(inputs["wq_b"], np.float32)
    wkv_a = np.asarray(inputs["wkv_a"], np.float32)
    w_kva_ln = np.asarray(inputs["w_kva_ln"], np.float32)
    wk_b = np.asarray(inputs["wk_b"], np.float32)
    wv_b = np.asarray(inputs["wv_b"], np.float32)
    wo_w = np.asarray(inputs["wo"], np.float32)
    rotary_sin = np.asarray(inputs["rotary_sin"], np.float32)
    rotary_cos = np.asarray(inputs["rotary_cos"], np.float32)

    def strips(wT, n_out_chunks, ko_chunks, w_per=128):
        # wT: [K, W] -> [n_out_chunks, 128, ko_chunks, w_per]
        K, W = wT.shape
        a = wT.reshape(ko_chunks, 128, n_out_chunks, w_per)
        return np.ascontiguousarray(a.transpose(2, 1, 0, 3)).astype(BF_NP)

    wqaT = wq_a.T                                             # [HID, QLR]
    wqa_pre = strips(wqaT, QLR_T, HID_T)                      # [12,128,16,128]

    kv_perm = (list(range(KVLR))
               + [KVLR + 2 * i for i in range(ROPE // 2)]
               + [KVLR + 2 * i + 1 for i in range(ROPE // 2)])
    wkvaT = wkv_a[kv_perm, :].T                               # [HID, 576]
    wkva_pre = strips(wkvaT[:, :KVLR], KVLR_T, HID_T)         # [4,128,16,128]
    wkvar_pre = np.ascontiguousarray(
        wkvaT[:, KVLR:].reshape(HID_T, 128, ROPE).transpose(1, 0, 2)
    ).astype(BF_NP)                                           # [128,16,64]

    wq_b_eff = wq_b * w_qa_ln[None, :]
    wk_b_eff = wk_b * w_kva_ln[None, :]
    wv_b_eff = wv_b * w_kva_ln[None, :]

    per_group = []
    for g in range(2):
        heads = range(g * NH, (g + 1) * NH)
        qperm = [hh * (NOPE + ROPE) + dd for hh in heads for dd in range(NOPE)]
        for hh in heads:
            qperm += [hh * (NOPE + ROPE) + NOPE + 2 * i for i in range(ROPE // 2)]
            qperm += [hh * (NOPE + ROPE) + NOPE + 2 * i + 1
                      for i in range(ROPE // 2)]
        wqbT = wq_b_eff[qperm, :].T                           # [QLR, 1536]
        wqb_pre = strips(wqbT, QF_T, QLR_T)                   # [12,128,12,128]
        cols = [hh * NOPE + dd for hh in heads for dd in range(NOPE)]
        wkbT = wk_b_eff[cols, :].T                            # [KVLR, 1024]
        wkb_pre = np.ascontiguousarray(
            wkbT.reshape(KVLR_T, 128, NH * NOPE)).astype(BF_NP)
        wvbT = wv_b_eff[cols, :].T
        wvb_pre = np.ascontiguousarray(
            wvbT.reshape(KVLR_T, 128, NH * VD)).astype(BF_NP)
        woT = wo_w[:, cols].T                                 # [1024, HID]
        wo_pre = np.zeros((4, 128, NH, 512), np.float32)
        for hc in range(4):
            blk = woT[:, hc * 512:(hc + 1) * 512]             # [1024, 512]
            wo_pre[hc] = blk.reshape(NH, 128, 512).transpose(1, 0, 2)
        wo_pre = wo_pre.astype(BF_NP)
        per_group.append((wqb_pre, wkb_pre, wvb_pre, wo_pre))

    cosT = np.ascontiguousarray(rotary_cos.T)                 # [32, S]
    sinT = np.ascontiguousarray(rotary_sin.T)
    c128 = np.tile(cosT, (4, 1)).astype(BF_NP)                # [128, S]
    s128 = np.tile(sinT, (4, 1)).astype(BF_NP)

    pswm = np.zeros((128, 128), np.float32)
    for blk in (0, 64):
        for i in range(32):
            pswm[blk + i, blk + 32 + i] = 1.0     # t_b even-row -> odd out
            pswm[blk + 32 + i, blk + i] = -1.0    # t_b odd-row  -> even out
    pswm = pswm.astype(BF_NP)
    id128 = np.eye(128, dtype=np.float32).astype(BF_NP)

    kq = np.arange(128)[:, None]
    qq = np.arange(QB)[None, :]
    masks = np.stack([
        np.where(kq <= qq, 0.0, NEG).astype(np.float32),
        np.where(kq + 128 <= qq, 0.0, NEG).astype(np.float32),
    ])
    ones_col = np.ones((128, 1), np.float32).astype(BF_NP)
    ones_row = np.ones((1, 128), np.float32).astype(BF_NP)
    ones_sq = np.ones((128, 128), np.float32).astype(BF_NP)

    # x per sequence -> [128, 16, 1024] (p, ko, t)
    xp_b = []
    for b in range(B):
        xb = x[b * S:(b + 1) * S].T                           # [HID, S]
        xp_b.append(np.ascontiguousarray(
            xb.reshape(HID_T, 128, S).transpose(1, 0, 2)).astype(BF_NP))

    in_maps = []
    for c in range(8):
        b, g = c // 2, c % 2
        wqb_pre, wkb_pre, wvb_pre, wo_pre = per_group[g]
        in_maps.append(dict(
            xp=xp_b[b],
            wqa=np.ascontiguousarray(wqa_pre[g * QLOC:(g + 1) * QLOC]),
            wqb=wqb_pre,
            wkva=np.ascontiguousarray(wkva_pre[g * KLOC:(g + 1) * KLOC]),
            wkvar=wkvar_pre, wkb=wkb_pre, wvb=wvb_pre, wo=wo_pre,
            c128=c128, s128=s128, psw=pswm, id128=id128, masks=masks,
            ones_col=ones_col, ones_row=ones_row, ones_sq=ones_sq))
    return in_maps


def assemble_output(results: list[dict]) -> np.ndarray:
    outs = []
    for b in range(B):
        outs.append(results[2 * b]["out"].astype(np.float32)
                    + results[2 * b + 1]["out"].astype(np.float32))
    return np.concatenate(outs, axis=0)


# ---------------------------------------------------------------------------
# Harness entry point: full inputs in, full output out.
# ---------------------------------------------------------------------------

_NC_CACHE = []


def _get_nc():
    if not _NC_CACHE:
        _NC_CACHE.append(build_nc())
    return _NC_CACHE[0]


def kernel(_profile=False, **inputs) -> np.ndarray:
    """MLA attention on 8 NeuronCores: 4-way data-parallel over sequences x
    2-way tensor-parallel over heads. Takes full (unsharded) inputs, returns
    the full [4096, 2048] float32 output."""
    from concourse.bass_utils import run_bass_kernel_spmd

    seqstarts = np.asarray(inputs["seqstarts"])
    b = seqstarts.shape[0] - 1
    assert b == B and np.all(np.diff(seqstarts) == S), (
        "kernel compiled for 4 uniform sequences of 1024 tokens")

    nc = _get_nc()
    in_maps = prepare_inputs(inputs)
    kwargs = {}
    if _profile:
        _install_ntff_hook()
        kwargs = dict(trace=True, trace_cores=list(range(8)))
    res = run_bass_kernel_spmd(nc, in_maps, list(range(8)), **kwargs)
    out = assemble_output(res.results).astype(np.float32)
    if _profile:
        return out, res
    return out


def _install_ntff_hook():
    """The agent image lacks antenv.axon_hooks; reconstruct the NTFF profile
    hook via ctypes so run_bass_kernel_spmd(trace=True) works (profiling-only
    path, used by test.py)."""
    import types
    if 'antenv.axon_hooks' in sys.modules:
        return
    try:
        from trn_agent_boot.trn_boot import _ntff_profile_via_ctypes
        hook = _ntff_profile_via_ctypes('/opt/axon/libaxon_pjrt.so')
    except Exception:
        hook = None
    mod = types.ModuleType('antenv.axon_hooks')
    mod.get_axon_ntff_profile_hook = lambda: hook
    sys.modules['antenv.axon_hooks'] = mod


# revision 62
# speedup vs baseline: 1.1612x; 1.0591x over previous
"""MLA attention TRN2 kernel: 4-way data-parallel (sequences) x 2-way
tensor-parallel (heads). Each core: 1 sequence (1024 tokens), 8 heads.

v2: all matmuls in bf16 (fp32r runs in fp32-HIGH PE mode at ~1.3 cyc/row
with slow non-FWL weight loads; bf16 gets 1.0 cyc/row + fast weight
load). RoPE done as batched full-width DVE muls + PE signed-swap
matmuls instead of 32-row DVE ops. A warmup matmul stream at program
start flips the PE HAM clock gate to 2.4 GHz while the x DMA streams.
Weights are pre-laid out on host into SBUF tile shapes so all weight
DMAs are contiguous.

Layout convention: features on partitions, tokens on free axis; scores
computed transposed [k, q] so softmax sums use PE ones-matmuls and no
transposes are needed anywhere. Softmax denominator folded into avT via
per-(head, qblock-pair) broadcast multiply.
"""
import sys
sys.path.insert(0, '/opt/trn_rl_repo')

import math
import numpy as np

import concourse.bass as bass
import concourse.tile as tile
from concourse.tile_rust import add_dep_helper
from concourse import bacc, mybir

BF = mybir.dt.bfloat16
F32 = mybir.dt.float32
AF = mybir.ActivationFunctionType

H = 16
NH = 8            # heads per core
NOPE = 128
ROPE = 64
VD = 128
HID = 2048
QLR = 1536
KVLR = 512
B = 4
S = 1024
EPS = 1e-6
SCALE = 1.0 / math.sqrt(NOPE + ROPE)
NEG = -1.0e30

HID_T = HID // 128    # 16
QLR_T = QLR // 128    # 12
KVLR_T = KVLR // 128  # 4
QLOC = QLR_T // 2     # 6 u strips computed locally (feature-split pair)
KLOC = KVLR_T // 2    # 2 ckv strips computed locally
CC_GROUPS = [[0, 1], [2, 3], [4, 5], [6, 7]]
QF = NH * (NOPE + ROPE)   # 1536 q features per core
QF_T = QF // 128          # 12 (chunks 0..7 nope, 8..11 rope)
NTC = S // 128            # 8 token chunks
QB = 256                  # query block
NQB = S // QB             # 4
N_WARM = 24               # warmup matmuls (keep HAM busy >3.4us)


def build_nc():
    nc = bacc.Bacc("TRN2", target_bir_lowering=False, debug=False, num_devices=8)

    xp = nc.dram_tensor("xp", [128, HID_T, S], BF, kind="ExternalInput")
    wqa = nc.dram_tensor("wqa", [QLOC, 128, HID_T, 128], BF, kind="ExternalInput")
    wqb = nc.dram_tensor("wqb", [QF_T, 128, QLR_T, 128], BF, kind="ExternalInput")
    wkva = nc.dram_tensor("wkva", [KLOC, 128, HID_T, 128], BF, kind="ExternalInput")
    wkvar = nc.dram_tensor("wkvar", [128, HID_T, ROPE], BF, kind="ExternalInput")
    wkb = nc.dram_tensor("wkb", [KVLR_T, 128, NH * NOPE], BF, kind="ExternalInput")
    wvb = nc.dram_tensor("wvb", [KVLR_T, 128, NH * VD], BF, kind="ExternalInput")
    wo = nc.dram_tensor("wo", [4, 128, NH, 512], BF, kind="ExternalInput")
    c128 = nc.dram_tensor("c128", [128, S], BF, kind="ExternalInput")
    s128 = nc.dram_tensor("s128", [128, S], BF, kind="ExternalInput")
    psw = nc.dram_tensor("psw", [128, 128], BF, kind="ExternalInput")
    id128 = nc.dram_tensor("id128", [128, 128], BF, kind="ExternalInput")
    masks = nc.dram_tensor("masks", [2, 128, QB], F32, kind="ExternalInput")
    ones_col = nc.dram_tensor("ones_col", [128, 1], BF, kind="ExternalInput")
    ones_row = nc.dram_tensor("ones_row", [1, 128], BF, kind="ExternalInput")
    ones_sq = nc.dram_tensor("ones_sq", [128, 128], BF, kind="ExternalInput")
    out = nc.dram_tensor("out", [S, HID], BF, kind="ExternalOutput")
    # pair-exchange scratch: u strips streamed as 3 gathers of 2 strips each
    # (flat row-major; last one carries the z2q partial row), ckv as one.
    du_loc = [nc.dram_tensor(f"du_loc{i}", [256, S],
                             BF, kind="Internal") for i in range(3)]
    du_gat = [nc.dram_tensor(f"du_gat{i}", [2, 256, S],
                             BF, kind="Internal") for i in range(3)]
    dz_loc = nc.dram_tensor("dz_loc", [1, S], BF, kind="Internal")
    dz_gat = nc.dram_tensor("dz_gat", [2, 1, S], BF, kind="Internal")
    dk_loc = nc.dram_tensor("dk_loc", [KLOC * 128 + 1, S], BF, kind="Internal")
    dk_gat = nc.dram_tensor("dk_gat", [2, KLOC * 128 + 1, S], BF, kind="Internal")

    with tile.TileContext(nc, pool_alloc_mode="queue") as tc:
        build_body(tc, xp=xp, wqa=wqa, wqb=wqb, wkva=wkva, wkvar=wkvar,
                   wkb=wkb, wvb=wvb, wo=wo, c128=c128, s128=s128, psw=psw,
                   id128=id128, masks=masks, ones_col=ones_col,
                   ones_row=ones_row, ones_sq=ones_sq, out=out,
                   du_loc=du_loc, du_gat=du_gat, dk_loc=dk_loc, dk_gat=dk_gat,
                   dz_loc=dz_loc, dz_gat=dz_gat)
    nc.compile()
    return nc


def build_body(tc, *, xp, wqa, wqb, wkva, wkvar, wkb, wvb, wo, c128, s128,
               psw, id128, masks, ones_col, ones_row, ones_sq, out,
               du_loc, du_gat, dk_loc, dk_gat, dz_loc, dz_gat):
    from contextlib import ExitStack
    nc = tc.nc

    with (
        tc.tile_pool(name="const", bufs=1) as pconst,
        tc.tile_pool(name="ckv", bufs=1) as pckv,
    ):
        ones_c = pconst.tile([128, 1], BF, name="ones_c")
        nc.sync.dma_start(ones_c[:], ones_col[:])
        ones_r = pconst.tile([1, 128], BF, name="ones_r")
        nc.sync.dma_start(ones_r[:], ones_row[:])
        ones_s = pconst.tile([128, 128], BF, name="ones_s")
        nc.sync.dma_start(ones_s[:], ones_sq[:])
        cos_sb = pconst.tile([128, S], BF, name="cos_sb")
        nc.sync.dma_start(cos_sb[:], c128[:])
        sin_sb = pconst.tile([128, S], BF, name="sin_sb")
        nc.sync.dma_start(sin_sb[:], s128[:])
        psw_sb = pconst.tile([128, 128], BF, name="psw_sb")
        nc.sync.dma_start(psw_sb[:], psw[:])
        id_sb = pconst.tile([128, 128], BF, name="id_sb")
        nc.sync.dma_start(id_sb[:], id128[:])
        mask_sb = []
        for i in range(2):
            m = pconst.tile([128, QB], F32, name=f"mask{i}")
            nc.sync.dma_start(m[:], masks[i])
            mask_sb.append(m)
        kpe2 = pconst.tile([128, S], BF, name="kpe2")
        eps_t = pconst.tile([1, 1], F32, name="eps_t")
        nc.vector.memset(eps_t[:], EPS)

        ckv = [pckv.tile([128, S], BF, name=f"ckv{i}") for i in range(KVLR_T)]

        # ---- HAM warmup: keep PE busy while x/wqa DMAs stream ----
        with tc.tile_pool(name="pwarm", bufs=1, space="PSUM") as ppw:
            wps = ppw.tile([128, 512], F32, name="warm_ps")
            for i in range(N_WARM):
                nc.tensor.matmul(wps[:], ones_s[:], cos_sb[:, 0:512],
                                 start=True, stop=True, skip_group_check=True)

        # ============ Phases: q_a -> kv_a (norms overlap) -> q_b ============
        es_u = ExitStack()
        pu = es_u.enter_context(tc.tile_pool(name="u", bufs=1))
        es_qbc = ExitStack()
        pqbc = es_qbc.enter_context(tc.tile_pool(name="qbc", bufs=1))
        qbc = [pqbc.tile([128, 512], BF, name=f"qbcn{i}") for i in range(2)]
        es_x = ExitStack()
        px = es_x.enter_context(tc.tile_pool(name="xsb", bufs=1))

        xsb = px.tile([128, HID_T, S], BF, name="xsb")
        prev_x = {0: None, 1: None}
        for kb in range(4):
            eng = nc.gpsimd if kb % 2 == 0 else nc.sync
            dma = eng.dma_start(xsb[:, kb * 4:(kb + 1) * 4, :],
                                xp[:, kb * 4:(kb + 1) * 4, :])
            if prev_x[kb % 2] is not None:
                add_dep_helper(dma.ins, prev_x[kb % 2].ins, sync=True,
                               reason="serialize x waves so early tiles land first")
            prev_x[kb % 2] = dma
        u = [pu.tile([128, S], BF, name=f"u{i}") for i in range(QLR_T)]

        with (
            tc.tile_pool(name="wstripA", bufs=4) as pwA,
            tc.tile_pool(name="sqA", bufs=4) as psq,
            tc.tile_pool(name="normA", bufs=2) as pnorm,
            tc.tile_pool(name="ppmain", bufs=4, space="PSUM") as ppm,
            tc.tile_pool(name="ppz2", bufs=2, space="PSUM") as ppz2,
            tc.tile_pool(name="ppbc", bufs=2, space="PSUM") as ppbc,
        ):
            # ---- kv_a first: local 2 ckv strips + k_pe strip (both cores);
            # its small gather then hides under q_a ----
            z2k = [ppz2.tile([1, 512], F32, name=f"z2k{h}", tag="z2")
                   for h in range(2)]
            kv4 = pqbc.tile([64, S], BF, name="kv4")
            ckstage = [pu.tile([128, S], BF, name=f"ckst{i}")
                       for i in range(KLOC)]
            zkstage = pnorm.tile([1, S], BF, name="zkstage", tag="zst", bufs=1)
            k_dmas = []
            pend = []
            with nc.named_scope("kv_a"):
                for m in range(KLOC):
                    ws = pwA.tile([128, HID_T, 128], BF, name=f"wkva_s{m}",
                                  tag="wstrip")
                    nc.sync.dma_start(ws[:], wkva[m])
                    for half in range(2):
                        sl = slice(half * 512, (half + 1) * 512)
                        ps = ppm.tile([128, 512], F32, name=f"pskv{m}_{half}",
                                      tag="main")
                        for ko in range(HID_T):
                            mmk = nc.tensor.matmul(
                                ps[:], ws[:, ko, :], xsb[:, ko, sl],
                                start=(ko == 0), stop=(ko == HID_T - 1))
                            if m == 0 and half == 0 and ko == 0:
                                gate_kva = mmk
                        cpk = nc.scalar.copy(ckstage[m][:, sl], ps[:])
                        sq = psq.tile([128, 512], BF, name=f"sqk{m}_{half}",
                                      tag="sq")
                        nc.scalar.activation(sq[:], ps[:], AF.Square)
                        for fn in pend:
                            fn()
                        pend = []
                        pend.append(
                            lambda sq=sq, half=half, m=m: nc.tensor.matmul(
                                z2k[half][:], ones_c[:], sq[:],
                                start=(m == 0), stop=(m == KLOC - 1),
                                skip_group_check=True))
                    k_dmas.append(nc.gpsimd.dma_start(
                        dk_loc[m * 128:(m + 1) * 128, :], ckstage[m][:]))
                for fn in pend:
                    fn()
                pend = []
                for half in range(2):
                    sl = slice(half * 512, (half + 1) * 512)
                    nc.scalar.copy(zkstage[:, sl], z2k[half][:])
                k_dmas.append(
                    nc.gpsimd.dma_start(dk_loc[KLOC * 128:, :], zkstage[:]))
                cck = nc.gpsimd.collective_compute(
                    "AllGather", mybir.AluOpType.bypass,
                    replica_groups=CC_GROUPS,
                    ins=[dk_loc[:]], outs=[dk_gat[:]])
                for dma in k_dmas:
                    add_dep_helper(cck.ins, dma.ins, sync=True,
                                   reason="gather after ckv stage DMAs")
                for ko in range(KVLR_T):
                    g, i = ko // KLOC, ko % KLOC
                    dma = nc.gpsimd.dma_start(
                        ckv[ko][:], dk_gat[g, i * 128:(i + 1) * 128, :])
                    add_dep_helper(dma.ins, cck.ins, sync=True,
                                   reason="read ckv after gather")
                zkg = [pnorm.tile([1, S], BF, name=f"zkg{g}", tag=f"zkg{g}",
                                  bufs=1) for g in range(2)]
                for g in range(2):
                    dma = nc.gpsimd.dma_start(zkg[g][:],
                                              dk_gat[g, KLOC * 128:, :])
                    add_dep_helper(dma.ins, cck.ins, sync=True,
                                   reason="read z2k after gather")
                zkt = pnorm.tile([1, S], F32, name="zkt", tag="zkt", bufs=1)
                nc.vector.tensor_add(zkt[:], zkg[0][:], zkg[1][:])

            # ---- q_a: local 6 u strips; streamed pairwise AllGathers ----
            z2q = [ppz2.tile([1, 512], F32, name=f"z2q{h}", tag="z2")
                   for h in range(2)]
            ustage = [pu.tile([128, S], BF, name=f"ust{i}")
                      for i in range(QLOC)]
            zqstage = pnorm.tile([1, S], BF, name="zqstage", tag="zst", bufs=1)
            u_dmas = []
            zqg = [pqbc.tile([1, S], BF, name=f"zqg{g}") for g in range(2)]
            zqt = pqbc.tile([1, S], F32, name="zqt")

            def u_gather(cc_idx):
                # gather strips (2*cc_idx, 2*cc_idx+1)
                cc = nc.gpsimd.collective_compute(
                    "AllGather", mybir.AluOpType.bypass,
                    replica_groups=CC_GROUPS,
                    ins=[du_loc[cc_idx][:]], outs=[du_gat[cc_idx][:]])
                for dma in u_dmas:
                    add_dep_helper(cc.ins, dma.ins, sync=True,
                                   reason="gather after u stage DMAs")
                u_dmas.clear()
                for g in range(2):
                    for i in range(2):
                        ko = g * QLOC + cc_idx * 2 + i
                        dma = nc.gpsimd.dma_start(
                            u[ko][:],
                            du_gat[cc_idx][g, i * 128:(i + 1) * 128, :])
                        add_dep_helper(dma.ins, cc.ins, sync=True,
                                       reason="read u after gather")

            with nc.named_scope("q_a"):
                for m in range(QLOC):
                    ws = pwA.tile([128, HID_T, 128], BF, name=f"wqa_s{m}",
                                  tag="wstrip")
                    dma = nc.sync.dma_start(ws[:], wqa[m])
                    add_dep_helper(dma.ins, gate_kva.ins, sync=True,
                                   reason="stage wqa DMA after kv_a starts")
                    for half in range(2):
                        sl = slice(half * 512, (half + 1) * 512)
                        ps = ppm.tile([128, 512], F32, name=f"psu{m}_{half}",
                                      tag="main")
                        for ko in range(HID_T):
                            mmq = nc.tensor.matmul(
                                ps[:], ws[:, ko, :], xsb[:, ko, sl],
                                start=(ko == 0), stop=(ko == HID_T - 1))
                            if m == 0 and half == 0 and ko == 0:
                                gate_qa = mmq
                        cp = nc.scalar.copy(ustage[m][:, sl], ps[:])
                        sq = psq.tile([128, 512], BF, name=f"squ{m}_{half}",
                                      tag="sq")
                        nc.scalar.activation(sq[:], ps[:], AF.Square)
                        for fn in pend:
                            fn()
                        pend = []
                        pend.append(
                            lambda sq=sq, half=half, m=m: nc.tensor.matmul(
                                z2q[half][:], ones_c[:], sq[:],
                                start=(m == 0), stop=(m == QLOC - 1),
                                skip_group_check=True))
                    u_dmas.append(nc.gpsimd.dma_start(
                        du_loc[m // 2][(m % 2) * 128:(m % 2 + 1) * 128, :],
                        ustage[m][:]))
                    if m in (1, 3, 5):
                        u_gather(m // 2)
                for fn in pend:
                    fn()
                pend = []
                # z2q partials ride their own tiny gather so the u strips
                # aren't held back behind the z2q psum stop
                for half in range(2):
                    sl = slice(half * 512, (half + 1) * 512)
                    nc.scalar.copy(zqstage[:, sl], z2q[half][:])
                zdma = nc.gpsimd.dma_start(dz_loc[:], zqstage[:])
                ccz = nc.gpsimd.collective_compute(
                    "AllGather", mybir.AluOpType.bypass,
                    replica_groups=CC_GROUPS,
                    ins=[dz_loc[:]], outs=[dz_gat[:]])
                add_dep_helper(ccz.ins, zdma.ins, sync=True,
                               reason="z gather after stage")
                for g in range(2):
                    dma = nc.gpsimd.dma_start(zqg[g][:], dz_gat[g])
                    add_dep_helper(dma.ins, ccz.ins, sync=True,
                                   reason="read z2q after gather")
                nc.vector.tensor_add(zqt[:], zqg[0][:], zqg[1][:])

            # ---- k_pe strip (independent of the gathers: fills the
            # cck latency window right after q_a) ----
            with nc.named_scope("kv_rope"):
                ws = pwA.tile([128, HID_T, ROPE], BF, name="wkva_s4",
                              tag="wstrip")
                nc.sync.dma_start(ws[:], wkvar[:])
                for half in range(2):
                    sl = slice(half * 512, (half + 1) * 512)
                    ps = ppm.tile([128, 512], F32, name=f"pskvr_{half}",
                                  tag="main")
                    for ko in range(HID_T):
                        nc.tensor.matmul(
                            ps[:ROPE, :], ws[:, ko, :], xsb[:, ko, sl],
                            start=(ko == 0), stop=(ko == HID_T - 1))
                    nc.scalar.copy(kv4[:, sl], ps[:ROPE, :])

            # ---- ckv rmsnorm scale + k_pe RoPE (PE signed-swap) ----
            with nc.named_scope("ckv_norm"):
                for half in range(2):
                    sl = slice(half * 512, (half + 1) * 512)
                    rt = pnorm.tile([1, 512], F32, name=f"krt{half}", tag="rt", bufs=1)
                    nc.scalar.activation(rt[:], zkt[:, sl], AF.Sqrt,
                                         scale=1.0 / KVLR, bias=eps_t[:])
                    rr = pnorm.tile([1, 512], BF, name=f"krr{half}", tag="rr", bufs=1)
                    with nc.allow_low_precision(reason="bf16 rms factor"):
                        nc.vector.reciprocal(rr[:], rt[:])
                    pb = ppbc.tile([128, 512], F32, name=f"kpb{half}", tag="bc")
                    nc.tensor.matmul(pb[:], ones_r[:], rr[:])
                    bc = pnorm.tile([128, 512], BF, name=f"kbc{half}", tag="bc")
                    nc.scalar.copy(bc[:], pb[:])
                    for m in range(KVLR_T):
                        nc.vector.tensor_mul(ckv[m][:, sl], ckv[m][:, sl],
                                             bc[:])

        es_x.close()

        # ---- v_b early: covers the tail of the u AllGather ----
        es_v = ExitStack()
        pv = es_v.enter_context(tc.tile_pool(name="v", bufs=1, side="right"))
        v_sb = [[pv.tile([128, 512], BF, name=f"v{g}_{t}")
                 for t in range(NTC)] for g in range(2)]
        with (
            tc.tile_pool(name="wvb", bufs=1) as pwvb,
            tc.tile_pool(name="ppv", bufs=4, space="PSUM") as ppv,
        ):
            wvb_sb = [pwvb.tile([128, NH * VD], BF, name=f"wvb{i}")
                      for i in range(KVLR_T)]
            for ko in range(KVLR_T):
                dma = nc.sync.dma_start(wvb_sb[ko][:], wvb[ko])
                add_dep_helper(dma.ins, gate_qa.ins, sync=True,
                               reason="stage wvb DMA after q_a starts")
            with nc.named_scope("v_b"):
                for g in range(2):
                    for t in range(NTC):
                        ps = ppv.tile([128, 512], F32,
                                      name=f"psv{g}_{t}", tag="v")
                        for kk in range(KVLR_T):
                            nc.tensor.matmul(
                                ps[:],
                                ckv[kk][:, t * 128:(t + 1) * 128],
                                wvb_sb[kk][:, g * 512:(g + 1) * 512],
                                start=(kk == 0),
                                stop=(kk == KVLR_T - 1))
                        nc.scalar.copy(v_sb[g][t][:], ps[:])

        # ---- k_nope for all heads: fills the u-AllGather latency window ----
        es_kn = ExitStack()
        pkn = es_kn.enter_context(tc.tile_pool(name="kn", bufs=1, side="right"))
        kn_all = [pkn.tile([128, S], BF, name=f"kn{h}") for h in range(NH)]
        es_wkb = ExitStack()
        pwkb = es_wkb.enter_context(tc.tile_pool(name="wkb", bufs=1))
        wkb_sb = [pwkb.tile([128, NH * NOPE], BF, name=f"wkb{i}")
                  for i in range(KVLR_T)]
        for ko in range(KVLR_T):
            dma = nc.sync.dma_start(wkb_sb[ko][:], wkb[ko])
            add_dep_helper(dma.ins, gate_qa.ins, sync=True,
                           reason="stage wkb DMA after q_a starts")
        with tc.tile_pool(name="ppkn", bufs=2, space="PSUM") as ppkn:
            with nc.named_scope("k_b"):
                for hh in range(NH):
                    for half in range(2):
                        sl = slice(half * 512, (half + 1) * 512)
                        ps = ppkn.tile([128, 512], F32,
                                       name=f"pskn{hh}_{half}", tag="kn")
                        for kk in range(KVLR_T):
                            nc.tensor.matmul(
                                ps[:], wkb_sb[kk][:, hh * 128:(hh + 1) * 128],
                                ckv[kk][:, sl],
                                start=(kk == 0), stop=(kk == KVLR_T - 1))
                        nc.scalar.copy(kn_all[hh][:, sl], ps[:])
        es_wkb.close()

        # ---- u rmsnorm factor (gpsimd broadcast: no PE op blocks on the
        # z2q gather chain) + k_pe RoPE transform ----
        with (
            tc.tile_pool(name="normB", bufs=1) as pn2,
            tc.tile_pool(name="ppkpe", bufs=2, space="PSUM") as ppkp,
        ):
            with nc.named_scope("u_norm"):
                for half in range(2):
                    sl = slice(half * 512, (half + 1) * 512)
                    rt = pn2.tile([1, 512], F32, name=f"qrt{half}")
                    nc.scalar.activation(rt[:], zqt[:, sl], AF.Sqrt,
                                         scale=1.0 / QLR, bias=eps_t[:])
                    rr = pn2.tile([1, 512], BF, name=f"qrr{half}")
                    with nc.allow_low_precision(reason="bf16 rms factor"):
                        nc.vector.reciprocal(rr[:], rt[:])
                    nc.gpsimd.partition_broadcast(qbc[half][:], rr[:])

            # k_pe RoPE: elementwise cos/sin muls then PE pair-swap
            with nc.named_scope("kpe_rope"):
                kta = pn2.tile([64, S], BF, name="kta")
                nc.vector.tensor_mul(kta[:], kv4[:], cos_sb[0:64, :])
                ktb = pn2.tile([64, S], BF, name="ktb")
                nc.vector.tensor_mul(ktb[:], kv4[:], sin_sb[0:64, :])
                for half in range(2):
                    sl = slice(half * 512, (half + 1) * 512)
                    pk = ppkp.tile([64, 512], F32, name=f"pkpe{half}", tag="kp")
                    nc.tensor.matmul(pk[:], psw_sb[0:64, 0:64], ktb[:, sl],
                                     start=True, stop=False)
                    nc.tensor.matmul(pk[:], id_sb[0:64, 0:64], kta[:, sl],
                                     start=False, stop=True)
                    nc.scalar.copy(kpe2[0:64, sl], pk[:])
                nc.gpsimd.dma_start(kpe2[64:128, :], kpe2[0:64, :])

        # ---- q_b: qT = wqbT.T @ u -> [1536, S]; rope tiles first ----
        es_qT = ExitStack()
        pqT = es_qT.enter_context(tc.tile_pool(name="qT", bufs=1, side="right"))
        qT = [pqT.tile([128, S], BF, name=f"qTt{i}") for i in range(QF_T)]
        with (
            tc.tile_pool(name="wstripB", bufs=4) as pwB,
            tc.tile_pool(name="ropeB", bufs=2) as prope,
            tc.tile_pool(name="ppmainB", bufs=6, space="PSUM") as ppmB,
            tc.tile_pool(name="pprope", bufs=2, space="PSUM") as ppr,
        ):
            def rope_xform(m):
                # rope transform in place: qT[m] <- ta + Psw @ tb
                ta = prope.tile([128, S], BF, name=f"rta{m}", tag="ta")
                nc.vector.tensor_mul(ta[:], qT[m][:], cos_sb[:])
                tb = prope.tile([128, S], BF, name=f"rtb{m}", tag="tb")
                nc.vector.tensor_mul(tb[:], qT[m][:], sin_sb[:])
                for half in range(2):
                    sl = slice(half * 512, (half + 1) * 512)
                    pr = ppr.tile([128, 512], F32, name=f"psr{m}_{half}",
                                  tag="rope")
                    nc.tensor.matmul(pr[:], psw_sb[:], tb[:, sl],
                                     start=True, stop=False)
                    nc.tensor.matmul(pr[:], id_sb[:], ta[:, sl],
                                     start=False, stop=True)
                    nc.scalar.copy(qT[m][:, sl], pr[:])

            with nc.named_scope("q_b"):
                for mi, m in enumerate(list(range(NH, QF_T)) + list(range(NH))):
                    ws = pwB.tile([128, QLR_T, 128], BF, name=f"wqb_s{m}",
                                  tag="wstripB")
                    dma = nc.sync.dma_start(ws[:], wqb[m])
                    add_dep_helper(dma.ins, gate_kva.ins, sync=True,
                                   reason="stage wqb DMA after kv_a starts")
                    # contract over ko with the last-gathered strips at the
                    # end, so early q_b groups don't wait on the final CC
                    ko_order = [0, 1, 2, 3, 6, 7, 8, 9, 4, 5, 10, 11]
                    for half in range(2):
                        sl = slice(half * 512, (half + 1) * 512)
                        ps = ppmB.tile([128, 512], F32, name=f"psq{m}_{half}",
                                       tag="mainB")
                        for ki, ko in enumerate(ko_order):
                            mmb = nc.tensor.matmul(
                                ps[:], ws[:, ko, :], u[ko][:, sl],
                                start=(ki == 0), stop=(ki == QLR_T - 1))
                            if m == NH and half == 0 and ki == 0:
                                gate_qb = mmb
                        cp = nc.scalar.copy(qT[m][:, sl], ps[:])
                        nc.vector.tensor_mul(qT[m][:, sl], qT[m][:, sl],
                                             qbc[half][:])
                    # rope transform for tile 8+j deferred until nope strip j:
                    # its DVE chain then overlaps a full strip of matmuls
                    if 4 <= mi < 8:
                        rope_xform(NH + mi - 4)
        es_qbc.close()
        es_u.close()

        # ======================= attention =======================
        es_avT = ExitStack()
        pavT = es_avT.enter_context(tc.tile_pool(name="avT", bufs=1))
        avT = [pavT.tile([128, S], BF, name=f"avT{i}") for i in range(NH)]
        with (
            tc.tile_pool(name="pbuf", bufs=4) as ppbuf,
            tc.tile_pool(name="zbuf", bufs=2) as pzbuf,
            # ppsc opened last: its banks free first at pool close, so
            # o_proj's psums land on banks h7 is already done with
            tc.tile_pool(name="ppav", bufs=2, space="PSUM") as ppav,
            tc.tile_pool(name="ppz", bufs=2, space="PSUM") as ppz,
            tc.tile_pool(name="ppsc", bufs=4, space="PSUM") as ppsc,
        ):
            for h in range(NH):
                with nc.named_scope(f"attn_h{h}"):
                    amul = attention_head(
                        tc, h, qT=qT, kpe2=kpe2, kn=kn_all[h],
                        v_sb=v_sb, avT=avT,
                        mask_sb=mask_sb, ones_s=ones_s,
                        ppbuf=ppbuf, pzbuf=pzbuf,
                        ppsc=ppsc, ppav=ppav, ppz=ppz)
                    if h == 0:
                        gate_attn = amul

        # ======================= o_proj =======================
        es_qT.close()
        with (
            tc.tile_pool(name="wo", bufs=2) as pwo,
            tc.tile_pool(name="osb", bufs=3) as posb,
            tc.tile_pool(name="ppo", bufs=6, space="PSUM") as ppo,
        ):
            with nc.named_scope("o_proj"):
                for hc in range(4):
                    ws = pwo.tile([128, NH, 512], BF, name=f"wo_s{hc}",
                                  tag="wo")
                    dma = nc.sync.dma_start(ws[:], wo[hc])
                    add_dep_helper(dma.ins, gate_qb.ins, sync=True,
                                   reason="stage wo DMA after q_b starts")
                    for t in range(NTC):
                        ps = ppo.tile([128, 512], F32, name=f"pso{hc}_{t}",
                                      tag="o")
                        for kk in range(NH):
                            nc.tensor.matmul(
                                ps[:], avT[kk][:, t * 128:(t + 1) * 128],
                                ws[:, kk, :],
                                start=(kk == 0), stop=(kk == NH - 1))
                        ot = posb.tile([128, 512], BF, name=f"ot{hc}_{t}",
                                       tag="ot")
                        nc.scalar.copy(ot[:], ps[:])
                        nc.gpsimd.dma_start(
                            out[t * 128:(t + 1) * 128,
                                hc * 512:(hc + 1) * 512], ot[:])
        es_avT.close()
        es_qT.close()
        es_kn.close()
        es_v.close()


def attention_head(tc, h, *, qT, kpe2, kn, v_sb, avT, mask_sb,
                   ones_s, ppbuf, pzbuf, ppsc, ppav, ppz):
    nc = tc.nc

    # roped q_pe lives in qT[8 + h//2] rows [(h%2)*64 : (h%2)*64+64];
    # kpe2 has k_pe duplicated in both 64-row halves so stationary and
    # moving operands share a partition base.
    qpe_src = qT[NH + h // 2]
    base = (h % 2) * 64

    # Two query-block pairs, chunk loops interleaved for PE lookahead.
    # pair p covers qblocks (2p, 2p+1) at columns p*512..(p+1)*512.
    st = []
    for p in range(2):
        q0 = 2 * p
        nw = 2 * q0 + 2
        st.append(dict(
            q0=q0, nw=nw, nk=nw + 2,
            psl=slice(p * 512, (p + 1) * 512),
            nsl=slice(p * 512 + 256, p * 512 + 512),
            ps_av=ppav.tile([128, 512], F32, name=f"psav{h}_{p}", tag="av"),
            ps_z=ppz.tile([128, 512], F32, name=f"psz{h}_{p}", tag="z"),
        ))
    pend = []

    def chunk(p, kc):
        s = st[p]
        ksl = slice(kc * 128, (kc + 1) * 128)
        wide = kc < s["nw"]
        csl = s["psl"] if wide else s["nsl"]
        cn = 512 if wide else 256
        ps_s = ppsc.tile([128, 512], F32, name=f"pss{h}_{p}_{kc}", tag="sc")
        nc.tensor.matmul(ps_s[:, :cn], kn[:, ksl], qT[h][:, csl],
                         start=True, stop=False)
        nc.tensor.matmul(ps_s[:, :cn], kpe2[base:base + 64, ksl],
                         qpe_src[base:base + 64, csl],
                         start=False, stop=True)
        d = kc - 2 * s["q0"] if wide else kc - 2 * (s["q0"] + 1)
        if d >= 0:
            nc.vector.tensor_add(ps_s[:, 0:256], ps_s[:, 0:256],
                                 mask_sb[d][:])
        p_sb = ppbuf.tile([128, 512], BF, name=f"p{h}_{p}_{kc}", tag="p")
        nc.scalar.activation(p_sb[:, :cn], ps_s[:, :cn], AF.Exp, scale=SCALE)

        def avz():
            vt = v_sb[h // 4][kc][:, (h % 4) * 128:(h % 4 + 1) * 128]
            osl = slice(0, 512) if wide else slice(256, 512)
            nc.tensor.matmul(s["ps_av"][:, osl], vt, p_sb[:, :cn],
                             start=(kc == 0), stop=(kc == s["nk"] - 1),
                             skip_group_check=True)
            nc.tensor.matmul(s["ps_z"][:, osl], ones_s[:], p_sb[:, :cn],
                             start=(kc == 0), stop=(kc == s["nk"] - 1),
                             skip_group_check=True)
        return avz

    def flush():
        for fn in pend:
            fn()
        pend.clear()

    for kc in range(st[1]["nk"]):
        cur = []
        if kc < st[0]["nk"]:
            cur.append(chunk(0, kc))
        cur.append(chunk(1, kc))
        flush()
        pend.extend(cur)
    flush()

    mul0 = None
    for p in range(2):
        s = st[p]
        zr = pzbuf.tile([128, 512], F32, name=f"zr{h}_{p}", tag="zr")
        nc.vector.reciprocal_approx_fast(zr[:], s["ps_z"][:])
        mul = nc.vector.tensor_mul(avT[h][:, s["psl"]], s["ps_av"][:], zr[:])
        if mul0 is None:
            mul0 = mul
    return mul0


# ---------------------------------------------------------------------------
# Host-side prep
# ---------------------------------------------------------------------------

def prepare_inputs(inputs: dict) -> list[dict]:
    """Full problem inputs -> list of 8 per-core input maps (bf16 pre-laid)."""
    import ml_dtypes
    BF_NP = ml_dtypes.bfloat16

    x = np.asarray(inputs["x"], np.float32)
    wq_a = np.asarray(inputs["wq_a"], np.float32)
    w_qa_ln = np.asarray(inputs["w_qa_ln"], np.float32)
    wq_b = np.asarray(inputs["wq_b"], np.float32)
    wkv_a = np.asarray(inputs["wkv_a"], np.float32)
    w_kva_ln = np.asarray(inputs["w_kva_ln"], np.float32)
    wk_b = np.asarray(inputs["wk_b"], np.float32)
    wv_b = np.asarray(inputs["wv_b"], np.float32)
    wo_w = np.asarray(inputs["wo"], np.float32)
    rotary_sin = np.asarray(inputs["rotary_sin"], np.float32)
    rotary_cos = np.asarray(inputs["rotary_cos"], np.float32)

    def strips(wT, n_out_chunks, ko_chunks, w_per=128):
        # wT: [K, W] -> [n_out_chunks, 128, ko_chunks, w_per]
        K, W = wT.shape
        a = wT.reshape(ko_chunks, 128, n_out_chunks, w_per)
        return np.ascontiguousarray(a.transpose(2, 1, 0, 3)).astype(BF_NP)

    wqaT = wq_a.T                                             # [HID, QLR]
    wqa_pre = strips(wqaT, QLR_T, HID_T)                      # [12,128,16,128]

    kv_perm = (list(range(KVLR))
               + [KVLR + 2 * i for i in range(ROPE // 2)]
               + [KVLR + 2 * i + 1 for i in range(ROPE // 2)])
    wkvaT = wkv_a[kv_perm, :].T                               # [HID, 576]
    wkva_pre = strips(wkvaT[:, :KVLR], KVLR_T, HID_T)         # [4,128,16,128]
    wkvar_pre = np.ascontiguousarray(
        wkvaT[:, KVLR:].reshape(HID_T, 128, ROPE).transpose(1, 0, 2)
    ).astype(BF_NP)                                           # [128,16,64]

    wq_b_eff = wq_b * w_qa_ln[None, :]
    wk_b_eff = wk_b * w_kva_ln[None, :]
    wv_b_eff = wv_b * w_kva_ln[None, :]

    per_group = []
    for g in range(2):
        heads = range(g * NH, (g + 1) * NH)
        qperm = [hh * (NOPE + ROPE) + dd for hh in heads for dd in range(NOPE)]
        for hh in heads:
            qperm += [hh * (NOPE + ROPE) + NOPE + 2 * i for i in range(ROPE // 2)]
            qperm += [hh * (NOPE + ROPE) + NOPE + 2 * i + 1
                      for i in range(ROPE // 2)]
        wqbT = wq_b_eff[qperm, :].T                           # [QLR, 1536]
        wqb_pre = strips(wqbT, QF_T, QLR_T)                   # [12,128,12,128]
        cols = [hh * NOPE + dd for hh in heads for dd in range(NOPE)]
        wkbT = wk_b_eff[cols, :].T                            # [KVLR, 1024]
        wkb_pre = np.ascontiguousarray(
            wkbT.reshape(KVLR_T, 128, NH * NOPE)).astype(BF_NP)
        wvbT = wv_b_eff[cols, :].T
        wvb_pre = np.ascontiguousarray(
            wvbT.reshape(KVLR_T, 128, NH * VD)).astype(BF_NP)
        woT = wo_w[:, cols].T                                 # [1024, HID]
        wo_pre = np.zeros((4, 128, NH, 512), np.float32)
        for hc in range(4):
            blk = woT[:, hc * 512:(hc + 1) * 512]             # [1024, 512]
            wo_pre[hc] = blk.reshape(NH, 128, 512).transpose(1, 0, 2)
        wo_pre = wo_pre.astype(BF_NP)
        per_group.append((wqb_pre, wkb_pre, wvb_pre, wo_pre))

    cosT = np.ascontiguousarray(rotary_cos.T)                 # [32, S]
    sinT = np.ascontiguousarray(rotary_sin.T)
    c128 = np.tile(cosT, (4, 1)).astype(BF_NP)                # [128, S]
    s128 = np.tile(sinT, (4, 1)).astype(BF_NP)

    pswm = np.zeros((128, 128), np.float32)
    for blk in (0, 64):
        for i in range(32):
            pswm[blk + i, blk + 32 + i] = 1.0     # t_b even-row -> odd out
            pswm[blk + 32 + i, blk + i] = -1.0    # t_b odd-row  -> even out
    pswm = pswm.astype(BF_NP)
    id128 = np.eye(128, dtype=np.float32).astype(BF_NP)

    kq = np.arange(128)[:, None]
    qq = np.arange(QB)[None, :]
    masks = np.stack([
        np.where(kq <= qq, 0.0, NEG).astype(np.float32),
        np.where(kq + 128 <= qq, 0.0, NEG).astype(np.float32),
    ])
    ones_col = np.ones((128, 1), np.float32).astype(BF_NP)
    ones_row = np.ones((1, 128), np.float32).astype(BF_NP)
    ones_sq = np.ones((128, 128), np.float32).astype(BF_NP)

    # x per sequence -> [128, 16, 1024] (p, ko, t)
    xp_b = []
    for b in range(B):
        xb = x[b * S:(b + 1) * S].T                           # [HID, S]
        xp_b.append(np.ascontiguousarray(
            xb.reshape(HID_T, 128, S).transpose(1, 0, 2)).astype(BF_NP))

    in_maps = []
    for c in range(8):
        b, g = c // 2, c % 2
        wqb_pre, wkb_pre, wvb_pre, wo_pre = per_group[g]
        in_maps.append(dict(
            xp=xp_b[b],
            wqa=np.ascontiguousarray(wqa_pre[g * QLOC:(g + 1) * QLOC]),
            wqb=wqb_pre,
            wkva=np.ascontiguousarray(wkva_pre[g * KLOC:(g + 1) * KLOC]),
            wkvar=wkvar_pre, wkb=wkb_pre, wvb=wvb_pre, wo=wo_pre,
            c128=c128, s128=s128, psw=pswm, id128=id128, masks=masks,
            ones_col=ones_col, ones_row=ones_row, ones_sq=ones_sq))
    return in_maps


def assemble_output(results: list[dict]) -> np.ndarray:
    outs = []
    for b in range(B):
        outs.append(results[2 * b]["out"].astype(np.float32)
                    + results[2 * b + 1]["out"].astype(np.float32))
    return np.concatenate(outs, axis=0)


# ---------------------------------------------------------------------------
# Harness entry point: full inputs in, full output out.
# ---------------------------------------------------------------------------

_NC_CACHE = []


def _get_nc():
    if not _NC_CACHE:
        _NC_CACHE.append(build_nc())
    return _NC_CACHE[0]


def kernel(_profile=False, **inputs) -> np.ndarray:
    """MLA attention on 8 NeuronCores: 4-way data-parallel over sequences x
    2-way tensor-parallel over heads. Takes full (unsharded) inputs, returns
    the full [4096, 2048] float32 output."""
    from concourse.bass_utils import run_bass_kernel_spmd

    seqstarts = np.asarray(inputs["seqstarts"])
    b = seqstarts.shape[0] - 1
    assert b == B and np.all(np.diff(seqstarts) == S), (
        "kernel compiled for 4 uniform sequences of 1024 tokens")

    nc = _get_nc()
    in_maps = prepare_inputs(inputs)
    kwargs = {}
    if _profile:
        _install_ntff_hook()
        kwargs = dict(trace=True, trace_cores=list(range(8)))
    res = run_bass_kernel_spmd(nc, in_maps, list(range(8)), **kwargs)
    out = assemble_output(res.results).astype(np.float32)
    if _profile:
        return out, res
    return out


def _install_ntff_hook():
    """The agent image lacks antenv.axon_hooks; reconstruct the NTFF profile
    hook via ctypes so run_bass_kernel_spmd(trace=True) works (profiling-only
    path, used by test.py)."""
    import types
    if 'antenv.axon_hooks' in sys.modules:
        return
    try:
        from trn_agent_boot.trn_boot import _ntff_profile_via_ctypes
        hook = _ntff_profile_via_ctypes('/opt/axon/libaxon_pjrt.so')
    except Exception:
        hook = None
    mod = types.ModuleType('antenv.axon_hooks')
    mod.get_axon_ntff_profile_hook = lambda: hook
    sys.modules['antenv.axon_hooks'] = mod
